# revision 18
# baseline (speedup 1.0000x reference)
"""MultiHeadAttention Trainium2 kernel — fp8 DoubleRow Q/K projections +
fp8 DoubleRow scores + dual-engine (ACT exact / DVE fast-exp) softmax +
bf16 attention/output path.

Core c: batch b=c//2, heads [(c%2)*8, (c%2)*8+8) (512-wide D_MODEL slice).
Host sums the two partial output projections per batch and adds bo.

Cost-model-driven design (TimelineSim charges matmuls out_free_size c/row;
bf16 = 1 c/row, fp8 DoubleRow = 0.5 c/row contracting 2x128 rows per
instruction; ACT = 0.83 ns/elem, DVE = 1.04 ns/elem from f32 psum):
  - Q/K projections fp8 DoubleRow (x1/x2/wq/wk host-quantized to fp8 in
    [64, 2pair, kc, .] layout; wq/wk columns host-permuted so psum
    partitions land in the scrambled (hmod4*32 + dmod32) layout that
    scores DoubleRow wants); the bias-add+quantize to fp8 qt8/kt8 runs on
    ACT (Identity+bias) or DVE (tensor_scalar) whichever is less loaded.
  - V projection bf16 (from a separate bf16 copy of x2): fp8 V error
    (~2.5%/elem) passes through attention averaging undamped and would
    blow the 2e-2 gate; bf16 V is ~0.2%.
  - scores per head = ONE DoubleRow matmul [32,2,128]x[32,2,512] ->
    psum [128 k, 512 q].
  - exp: split between ACT (exact activation, ~1.04us/tick) and DVE
    (Schraudolph fast-exp: i16 = round(score*16*log2e + 16248.5) written
    into the bf16 ex tile via .bitcast(int16) = piecewise-linear exp with
    ~1.8% rms / 4.2% max per-element error; quota-capped so total output
    error stays well under the 2e-2 gate). Both produce bf16 ex tiles;
    attn@V and den read them identically.
  - attn@V bf16 as before; denominators via 1-column ones matmuls.
  - normalize: one DVE reciprocal [128,8] per pair + ONE broadcast
    tensor_tensor (outp * rden with a stride-0 ap) -> ao bf16; xbar
    DMA-transpose -> aot.
  - output projection bf16; psum halves drained by ACT-Copy or DVE-copy
    (least-loaded), DMA'd to HBM from the gpsimd (Pool) queue -- Pool is
    otherwise idle and its SWDGE descriptor generation is free ACT/DVE
    time.
Emission is one flat software-pipelined stream over 256 (qb,pair,kc)
ticks as in the previous version: per tick: exp(t) on the engine with the
earlier modeled finish | loads | one EDF-scheduled heavy PE fill | deferred
attn@V from a bounded backlog (age-released, v-chunk-gated) | scores(t+2)
last. PSUM: scores 2x2 banks + outp 2x1 + den 1 + fill scratch 1 = 8.
"""

import sys

sys.path.insert(0, "/opt/trn_rl_repo")

from collections import defaultdict
from contextlib import ExitStack

import numpy as np
import concourse.bass as bass
import concourse.tile as tile
from concourse import bacc, mybir
from concourse.bass_utils import run_bass_kernel_spmd

B, S, D_IN, D_MODEL, H = 4, 2048, 1024, 1024, 16
DH = 64
HPC = 8
DS = 512
F32 = mybir.dt.float32
BF16 = mybir.dt.bfloat16
FP8 = mybir.dt.float8e4
I16 = mybir.dt.int16
DRow = mybir.MatmulPerfMode.DoubleRow
Exp = mybir.ActivationFunctionType.Exp
Ident = mybir.ActivationFunctionType.Identity
Copy = mybir.ActivationFunctionType.Copy
Mult = mybir.AluOpType.mult
Add = mybir.AluOpType.add

NKC = D_IN // 128  # 8
NSC = S // 128  # 16
QB = 512
SCALE = 1.0 / np.sqrt(DH)
# Schraudolph fast-exp on DVE: bf16 bits of e^(score*SCALE) ~=
# round(score * 128*SCALE*log2(e) + 127*128 - 7.5); -7.5 centers the
# piecewise-linear sawtooth (1.8% rms / 4.2% max, measured on HW).
AEXP = 16.0 * float(np.log2(np.e))
BEXP = 16256.0 - 7.5
DVE_EXP_MAX = 192  # max fast-exp half-ticks (error budget dial)

D_MIN = 2  # min backlog: released attn@V must be stale so it never gates PE
AGE = 10  # attn@V runs AGE ticks behind its exp
MAX_BACKLOG = 16
QLAG = 2  # producer-to-consumer deferral ticks (psum scratch quantize/drain)
NLAG = 2  # attn@V-end to normalize
TLAG = 4  # normalize to xbar transposes
EX_BUFS = 18


def _col_perm():
    """Permuted D-column order for wq/wk so proj psum partitions match the
    scores-DoubleRow layout: chunk c=(hslot,dj), partition p ->
    col = 64*(p//32 + 4*hslot) + 32*dj + p%32."""
    perm = np.empty(DS, np.int64)
    for c in range(4):
        hslot, dj = c // 2, c % 2
        for p in range(128):
            perm[c * 128 + p] = 64 * (p // 32 + 4 * hslot) + 32 * dj + p % 32
    return perm


def _kernel_body(nc, tc, aps):
    x1, x2f, x2b, wq8, wk8, wv, wo, bq, bk, bv, out = aps

    with ExitStack() as ctx:
        pers = ctx.enter_context(tc.tile_pool(name="pers", bufs=1))

        wq8_sb = pers.tile([64, 2, 2, NKC, 256], FP8)  # [p, half, j, kc, ci]
        wk8_sb = pers.tile([64, 2, 2, NKC, 256], FP8)
        wv_sb = pers.tile([128, NKC, DS], BF16)
        wo_sb = pers.tile([128, 4, D_MODEL], BF16)
        x2f_sb = pers.tile([64, 2, NKC, S], FP8)  # [p, j, kc, t]
        x2b_sb = pers.tile([128, NKC, S], BF16)
        qt8 = pers.tile([128, 2, 2, S], FP8)  # [p, hslot, dj, q]
        kt8 = pers.tile([128, 2, 2, S], FP8)
        v_sb = pers.tile([128, NSC, HPC, DH], BF16)
        aot = pers.tile([128, 4, S], BF16)
        bq_sb = pers.tile([128, 4], F32)
        bk_sb = pers.tile([128, 4], F32)
        bv_bc = pers.tile([128, DS], F32)
        ones = pers.tile([128, 1], BF16)

        px1 = ctx.enter_context(tc.tile_pool(name="px1", bufs=2))
        x1ts = {}

        nc.gpsimd.memset(ones[:, :], 1.0)
        nc.gpsimd.dma_start(
            out=bq_sb, in_=bq.rearrange("(c p) o -> p (c o)", p=128)
        )
        nc.gpsimd.dma_start(
            out=bk_sb, in_=bk.rearrange("(c p) o -> p (c o)", p=128)
        )
        nc.gpsimd.dma_start(
            out=bv_bc, in_=bv.rearrange("s o -> o s").to_broadcast([128, DS])
        )

        # ---- engine-clock model (for exp/quantize/drain placement) ----
        eng_clk = {"a": 0.0, "d": 0.0}
        C_EA, C_ED = 600.0, 720.0  # exp per half-tick (one par)
        C_QA, C_QD = 600.0, 670.0  # qk quantize [128,512]
        C_VB = 670.0  # v bias (DVE only)
        C_NRM = 880.0  # normalize per pair (DVE only)
        C_DA, C_DD = 600.0, 670.0  # oproj drain half
        dve_exp_used = [0]

        def pick_eng(ca, cd):
            if eng_clk["a"] + ca <= eng_clk["d"] + cd:
                eng_clk["a"] += ca
                return "a"
            eng_clk["d"] += cd
            return "d"

        def ld_x1(sq, eng):
            t = px1.tile([64, 2, NKC, QB], FP8, tag="x8", name=f"x1_{sq}")
            eng.dma_start(
                out=t,
                in_=x1[:, :, sq * QB : (sq + 1) * QB].rearrange(
                    "p (j c) s -> p j c s", j=2
                ),
            )
            x1ts[sq] = t

        def ld_x2f(sq, eng):
            eng.dma_start(
                out=x2f_sb[:, :, :, sq * QB : (sq + 1) * QB],
                in_=x2f[:, :, sq * QB : (sq + 1) * QB].rearrange(
                    "p (j c) s -> p j c s", j=2
                ),
            )

        def ld_x2b(sq, eng):
            eng.dma_start(
                out=x2b_sb[:, :, sq * QB : (sq + 1) * QB],
                in_=x2b.rearrange("(c p) s -> p c s", p=128)[
                    :, :, sq * QB : (sq + 1) * QB
                ],
            )

        # fp8 DoubleRow projection chunk c of quarter sq -> fp8 qt8/kt8
        # (columns host-permuted so psum partitions land in the
        # scores-DoubleRow layout); bias-add+fp8-quantize on the less
        # loaded of ACT/DVE.
        # Cross-engine consumers (quantize / bias / drain / normalize /
        # transpose) are NOT emitted right after their producer matmuls:
        # engines execute in order, so a dep-waiting op at the head of
        # ACT/DVE would stall the exp stream behind it. Instead they are
        # queued here and emitted 1+ ticks later, when the producer is
        # (nearly) done. Safe with single-buffer psum scratch because
        # deferred thunks run BEFORE the tick's fill job allocates.
        deferred = defaultdict(list)
        cur_tick = [0]

        def qk_chunk(psp, which, sq, c):
            w_sb, dst, b_sb = (
                (wq8_sb, qt8, bq_sb) if which == 1 else (wk8_sb, kt8, bk_sb)
            )
            xt = x1ts[sq] if which == 1 else x2f_sb
            h, c2 = c // 2, c % 2
            ps = psp.tile([128, QB], F32, tag="pp", name="qkp")
            for kc in range(NKC):
                if which == 1:
                    mov = xt[:, :, kc, :]
                else:
                    mov = xt[:, :, kc, sq * QB : (sq + 1) * QB]
                nc.tensor.matmul(
                    ps,
                    w_sb[:, h, :, kc, c2 * 128 : (c2 + 1) * 128],
                    mov,
                    start=(kc == 0),
                    stop=(kc == NKC - 1),
                    perf_mode=DRow,
                )
            dsts = dst[:, c // 2, c % 2, sq * QB : (sq + 1) * QB]

            def quant():
                if pick_eng(C_QA, C_QD) == "a":
                    nc.scalar.activation(
                        dsts, ps, Ident, bias=b_sb[:, c : c + 1]
                    )
                else:
                    nc.vector.tensor_scalar_add(dsts, ps, b_sb[:, c : c + 1])

            return quant

        def v_chunk(psp, sq, sc):
            ps = psp.tile([128, QB], F32, tag="pp", name="vp")
            for kc in range(NKC):
                nc.tensor.matmul(
                    ps,
                    x2b_sb[:, kc, sq * QB + sc * 128 : sq * QB + (sc + 1) * 128],
                    wv_sb[:, kc, :],
                    start=(kc == 0),
                    stop=(kc == NKC - 1),
                )

            def vbias():
                eng_clk["d"] += C_VB
                nc.vector.tensor_add(
                    v_sb[:, sq * 4 + sc, :, :],
                    ps.rearrange("p (h d) -> p h d", h=HPC),
                    bv_bc.rearrange("p (h d) -> p h d", h=HPC),
                )

            return vbias

        # ---- load streams ----
        # DMA transfers serialize on the modeled (exclusive) DMA device, so
        # order IS the schedule. Scalar carries only the loads the first
        # scores need; everything else on sync. ACT/DVE never issue DMAs
        # (their 667ns seq dispatch would stall the exp stream).
        dma_ready = {}
        _dma_clk = [2000.0]

        def dma_ns(total_bytes, elem):
            lat = 2.0 if elem < 512 else 1.0
            return total_bytes / elem / 16.0 * max(elem * lat / 22.5, 7.0)

        def _track(name, total_bytes, elem, emit):
            emit()
            _dma_clk[0] += dma_ns(total_bytes, elem) + 800.0
            dma_ready[name] = _dma_clk[0]

        KB = 1024
        _track("wk8h", 256 * KB, 4096,
               lambda: nc.scalar.dma_start(out=wk8_sb[:, 0], in_=wk8[:, 0, :].rearrange("p (j c i) -> p j c i", j=2, c=NKC)))
        _track("x2f0", 512 * KB, 512, lambda: ld_x2f(0, nc.scalar))
        _track("wq8h", 256 * KB, 4096,
               lambda: nc.sync.dma_start(out=wq8_sb[:, 0], in_=wq8[:, 0, :].rearrange("p (j c i) -> p j c i", j=2, c=NKC)))
        _track("x1q0", 512 * KB, 512, lambda: ld_x1(0, nc.sync))
        _track("x2f1", 512 * KB, 512, lambda: ld_x2f(1, nc.sync))
        _track("wv", 1024 * KB, 1024,
               lambda: nc.sync.dma_start(out=wv_sb, in_=wv.rearrange("(c p) o -> p c o", p=128)))
        _track("x2f2", 512 * KB, 512, lambda: ld_x2f(2, nc.sync))
        _track("x2b0", 1024 * KB, 1024, lambda: ld_x2b(0, nc.sync))
        _track("x2f3", 512 * KB, 512, lambda: ld_x2f(3, nc.sync))
        _track("x2b1", 1024 * KB, 1024, lambda: ld_x2b(1, nc.sync))
        _track("x2b2", 1024 * KB, 1024, lambda: ld_x2b(2, nc.sync))

        # startup projection chunks in their own psum pool. Dummy matmuls
        # first ramp the PE clock out of its cold p-state.
        warm = pers.tile([128, 512], BF16)
        nc.gpsimd.memset(warm[:, :], 0.0)
        with tc.tile_pool(name="psA", bufs=4, space="PSUM") as psA:
            wps = psA.tile([128, 512], F32, tag="pp", name="warmp")
            for i in range(8):
                nc.tensor.matmul(
                    wps[0:1, :],
                    ones[:, 0:1],
                    warm[:, :],
                    start=(i == 0),
                    stop=(i == 7),
                )
            qk_chunk(psA, 2, 0, 0)()
            qk_chunk(psA, 2, 0, 1)()
            qk_chunk(psA, 1, 0, 0)()
            qk_chunk(psA, 1, 0, 1)()

        # ---- attention pools ----
        attn_ctx = ctx.enter_context(ExitStack())
        psc = attn_ctx.enter_context(tc.tile_pool(name="psc", bufs=4, space="PSUM"))
        pso = attn_ctx.enter_context(tc.tile_pool(name="pso", bufs=1, space="PSUM"))
        pdn = attn_ctx.enter_context(tc.tile_pool(name="pdn", bufs=1, space="PSUM"))
        psp = attn_ctx.enter_context(tc.tile_pool(name="psp", bufs=2, space="PSUM"))
        pex = attn_ctx.enter_context(tc.tile_pool(name="pex", bufs=EX_BUFS))
        pao = attn_ctx.enter_context(tc.tile_pool(name="pao", bufs=2))
        prd = attn_ctx.enter_context(tc.tile_pool(name="prd", bufs=2))
        pot = attn_ctx.enter_context(tc.tile_pool(name="pot", bufs=2))

        ot_tiles = {}

        def oproj_half(mb, nt):
            if nt == 0:
                ot_tiles[mb] = pot.tile(
                    [128, D_MODEL], BF16, tag="ot", name="oti"
                )
            ot = ot_tiles[mb]
            ps = psp.tile([128, 512], F32, tag="pp", name="opp")
            for kc in range(4):
                nc.tensor.matmul(
                    ps,
                    aot[:, kc, mb * 128 : (mb + 1) * 128],
                    wo_sb[:, kc, nt * 512 : (nt + 1) * 512],
                    start=(kc == 0),
                    stop=(kc == 3),
                )

            def drain():
                if pick_eng(C_DA, C_DD) == "a":
                    nc.scalar.activation(
                        ot[:, nt * 512 : (nt + 1) * 512], ps, Copy
                    )
                else:
                    nc.vector.tensor_copy(ot[:, nt * 512 : (nt + 1) * 512], ps)
                if nt == 1:
                    nc.gpsimd.dma_start(
                        out=out[mb * 128 : (mb + 1) * 128, :], in_=ot
                    )

            return drain

        # ---- filler schedule: tick -> thunks ----
        # chunk c=(hslot,dj) of a quarter serves pairs 2*(c//2), 2*(c//2)+1.
        CQK, CV, COPH = 860, 1707, 860
        T0, TICK = 10000.0, 880.0

        def r2t(ns):
            return max(0, int((ns - T0) / TICK) + 1)

        # jobs: (deadline_tick, ready_tick, pe_cost, thunk, tag). One heavy
        # job per tick, earliest-deadline-first among ready jobs. qt/kt
        # chunks MUST land by their deadline (scores would otherwise read
        # uninitialized sbuf = a real race); v jobs may slip (attn@V release
        # is gated on the v chunk being emitted, the backlog absorbs it).
        jobs = []
        for s in range(1, 4):
            for c in range(2):
                jobs.append(
                    (4 * s - 4 + c, r2t(dma_ready[f"x2f{s}"]), CQK,
                     lambda s=s, c=c: qk_chunk(psp, 2, s, c), None, 4 * s)
                )
        # pair 2 (tick 32+4s) reads BOTH dj chunks (c2 and c3) of hslot 1
        # c2/c3 need the half-1 weight loads dispatched at fill ticks 3/4:
        # ready >= 6 also orders the emission after those dma_starts.
        for s in range(4):
            rd = r2t(dma_ready[f"x2f{s}"])
            jobs.append(
                (28 + 4 * s, max(rd, 6), CQK,
                 lambda s=s: qk_chunk(psp, 2, s, 2), None, 32 + 4 * s)
            )
            jobs.append(
                (29 + 4 * s, max(rd, 6), CQK,
                 lambda s=s: qk_chunk(psp, 2, s, 3), None, 32 + 4 * s)
            )
        for c in (2, 3):
            jobs.append(
                (26 + c, max(r2t(dma_ready["x1q0"]), 6), CQK,
                 lambda c=c: qk_chunk(psp, 1, 0, c), None, 32)
            )
        for s in range(4):
            rv = r2t(max(dma_ready.get(f"x2b{s}", 0.0), dma_ready["wv"])) \
                if s < 3 else 28
            for sc in range(4):
                dl = max(4 * s + sc + AGE - 1, rv)
                jobs.append(
                    (dl, rv, CV,
                     lambda s=s, sc=sc: v_chunk(psp, s, sc), ("v", 4 * s + sc),
                     None)
                )
        for sq in range(1, 4):
            for c in range(4):
                jobs.append(
                    (64 * sq - 4 + c if c < 2 else 64 * sq + 26 + c,
                     64 * (sq - 1) + 6, CQK,
                     lambda sq=sq, c=c: qk_chunk(psp, 1, sq, c), None,
                     64 * sq if c < 2 else 64 * sq + 32)
                )
        # oproj halves: the source qb's last transpose is emitted when its
        # final attn@V leaves the backlog (qb end + AGE)
        # aot for block qb is complete once pair3's transposes are emitted:
        # release(qb,3,15) ~ 64qb+63+AGE, + NLAG (norm) + TLAG (tps) + margin
        for mb in range(12):
            for nt in range(2):
                jobs.append(
                    (1000 + 2 * mb + nt,
                     64 * (mb // 4 + 1) + AGE + TLAG + 4, COPH,
                     lambda mb=mb, nt=nt: oproj_half(mb, nt), None, None)
                )
        jobs.sort(key=lambda j: (j[0], j[1]))
        # Precompute the EDF tick assignment; qt/kt jobs must land by their
        # deadline, v jobs gate attn@V release below.
        v_done_tick = {}
        _pending = list(range(len(jobs)))
        assigned = {}
        for t in range(256):
            pick = None
            for idx in _pending:
                if jobs[idx][0] <= t or jobs[idx][1] <= t:
                    pick = idx
                    break
            if pick is not None:
                assigned[t] = pick
                _pending.remove(pick)
                tag = jobs[pick][4]
                if tag and tag[0] == "v":
                    v_done_tick[tag[1]] = t
                rt = jobs[pick][5]
                if rt is not None:
                    assert t <= rt - 3, (t, rt)
        assert not _pending, f"{len(_pending)} jobs unassigned"
        assert len(v_done_tick) == 16
        # zero-cost emissions (loads) at fixed ticks
        fill = defaultdict(list)
        fill[2].append(lambda: ld_x2b(3, nc.sync))
        fill[3].append(
            lambda: nc.sync.dma_start(out=wk8_sb[:, 1], in_=wk8[:, 1, :].rearrange("p (j c i) -> p j c i", j=2, c=NKC))
        )
        fill[4].append(
            lambda: nc.sync.dma_start(out=wq8_sb[:, 1], in_=wq8[:, 1, :].rearrange("p (j c i) -> p j c i", j=2, c=NKC))
        )
        fill[5].append(lambda: ld_x1(1, nc.sync))
        for sq in range(2, 4):
            fill[64 * (sq - 1) + 2].append(lambda sq=sq: ld_x1(sq, nc.sync))
        fill[20].append(
            lambda: nc.sync.dma_start(
                out=wo_sb, in_=wo.rearrange("(c p) o -> p c o", p=128)
            )
        )

        # ---- flat pipelined attention stream ----
        TICKS = [
            (qb, pair, kc)
            for qb in range(4)
            for pair in range(4)
            for kc in range(NSC)
        ]
        sc_tiles = {}
        ex_tiles = {}
        state = {}

        def emit_sc(t):
            qb, pair, kc = TICKS[t]
            q0 = qb * QB
            tiles = []
            for par in range(2):
                scp = psc.tile([128, QB], F32, tag="sc", name="scp")
                h = 2 * pair + par
                hb = 32 * (h % 4)
                nc.tensor.matmul(
                    scp,
                    kt8[hb : hb + 32, h // 4, :, kc * 128 : (kc + 1) * 128],
                    qt8[hb : hb + 32, h // 4, :, q0 : q0 + QB],
                    start=True,
                    stop=True,
                    perf_mode=DRow,
                    tile_position=(hb, 0),
                )
                tiles.append(scp)
            sc_tiles[t] = tiles

        def emit_exp(t, ready_ns):
            expt = pex.tile([128, 2, QB], BF16, tag="ex", name="ext")
            for par, scp in enumerate(sc_tiles.pop(t)):
                fa = max(eng_clk["a"], ready_ns) + C_EA
                fd = max(eng_clk["d"], ready_ns) + C_ED
                if fd < fa and dve_exp_used[0] < DVE_EXP_MAX:
                    dve_exp_used[0] += 1
                    eng_clk["d"] = fd
                    nc.vector.tensor_scalar(
                        expt[:, par, :].bitcast(I16), scp, AEXP, BEXP, Mult, Add
                    )
                else:
                    eng_clk["a"] = fa
                    nc.scalar.activation(
                        expt[:, par, :], scp, Exp, scale=float(SCALE)
                    )
            ex_tiles[t] = expt

        def emit_av(t):
            qb, pair, kc = TICKS[t]
            expt = ex_tiles.pop(t)
            if kc == 0:
                state["outp"] = pso.tile(
                    [128, 2, 4, DH], F32, tag="acc", name="outp"
                )
                if pair == 0:
                    state["den"] = pdn.tile([128, 32], F32, tag="dn", name="den")
                    state["rden"] = prd.tile([128, 32], F32, tag="rd", name="rden")
            outp = state["outp"]
            den = state["den"]
            for par in range(2):
                h = 2 * pair + par
                for qc in range(4):
                    exs = expt[:, par, qc * 128 : (qc + 1) * 128]
                    first = kc == 0 and par == 0 and qc == 0
                    last = kc == NSC - 1 and par == 1 and qc == 3
                    nc.tensor.matmul(
                        outp[:, par, qc, :],
                        exs,
                        v_sb[:, kc, h, :],
                        start=first,
                        stop=last,
                    )
                    di = pair * 8 + par * 4 + qc
                    nc.tensor.matmul(
                        den[:, di : di + 1],
                        exs,
                        ones[:, :],
                        start=first,
                        stop=last,
                    )
            if kc == NSC - 1:
                q0 = qb * QB
                rden = state["rden"]
                # reciprocal inline: it must execute before the next pair's
                # den accumulation group opens in the same psum tensor
                rsl = rden[:, pair * 8 : (pair + 1) * 8]
                nc.vector.reciprocal(rsl, den[:, pair * 8 : (pair + 1) * 8])
                ao = pao.tile([128, 4, 2, DH], BF16, tag="ao", name="ao")
                nc.vector.tensor_tensor(
                    out=ao[:, :, :, :],
                    in0=outp.rearrange("p a b d -> p b a d"),
                    in1=rsl.rearrange("p (a b) -> p b a", a=2).to_broadcast(
                        [128, 4, 2, DH]
                    ),
                    op=Mult,
                )
                eng_clk["d"] += C_NRM

                def tps():
                    for qc in range(4):
                        nc.sync.dma_start_transpose(
                            aot[:, pair, q0 + qc * 128 : q0 + (qc + 1) * 128],
                            ao[:, qc, :, :],
                        )

                deferred[cur_tick[0] + TLAG].append(tps)

        # Greedy emission: track modeled PE/ACT/DVE clocks; defer attn@V
        # work (bounded backlog) and drain it age-based so the ex pool never
        # starves the exp stream. Never release an attn@V whose v chunk
        # hasn't been emitted yet (emission order defines dependency order).
        C_SC, C_AV = 213.0, 220.0
        pe_t = 5500.0  # first matmul lands after the startup DMA chain
        eng_clk["a"] = eng_clk["d"] = 7000.0
        sc_done = {}
        backlog = []

        emit_sc(0)
        sc_done[0] = pe_t = pe_t + C_SC
        emit_sc(1)
        sc_done[1] = pe_t = pe_t + C_SC
        for t in range(256):
            cur_tick[0] = t
            # deferred producers-consumers first: their inputs are ~done, and
            # queue position ahead of exp(t) lets vbias/quant unblock the
            # attn@V -> ex-pool chain instead of sitting behind a 1.2us exp
            for th in deferred.pop(t, ()):
                th()
            emit_exp(t, sc_done[t] + 100.0)
            backlog.append(t)
            for f in fill[t]:
                f()
            if t in assigned:
                dl, rd, cost, th, tag, rt = jobs[assigned[t]]
                post = th()
                pe_t += cost
                if post is not None:
                    # the quantize must be emitted before the scores reading
                    # its output region (emitted at tick rt-2, sc phase)
                    lag = QLAG if rt is None else max(1, min(QLAG, rt - 2 - t))
                    deferred[t + lag].append(post)
            # attn@V after the job: these small matmuls overlap the job's
            # psum-drain latency so back-to-back fills don't bubble PE.
            while backlog and (
                len(backlog) > MAX_BACKLOG
                or (len(backlog) > D_MIN and t - backlog[0] >= AGE)
            ):
                qbu, pairu, kcu = TICKS[backlog[0]]
                if qbu == 0 and v_done_tick[kcu] + QLAG > t:
                    break
                emit_av(backlog.pop(0))
                pe_t += C_AV
            # exp-gated score matmul last, so jobs/attn@V never sit behind
            # the gate in the PE queue
            if t + 2 < 256:
                emit_sc(t + 2)
                pe_t += C_SC
                sc_done[t + 2] = pe_t
        for u in backlog:
            cur_tick[0] += 1
            emit_av(u)
            for th in deferred.pop(cur_tick[0], ()):
                th()
        while deferred:
            t = min(deferred)
            cur_tick[0] = max(cur_tick[0], t)
            for th in deferred.pop(t):
                th()

        attn_ctx.close()

        # ---- tail: last output-projection blocks ----
        with tc.tile_pool(name="psD", bufs=4, space="PSUM") as psD, tc.tile_pool(
            name="potD", bufs=2
        ) as potD:
            for mb in range(12, 16):
                ot = potD.tile([128, D_MODEL], BF16, tag="ot", name="otd")
                for nt in range(2):
                    ps = psD.tile([128, 512], F32, tag="pf", name="opd")
                    for kc in range(4):
                        nc.tensor.matmul(
                            ps,
                            aot[:, kc, mb * 128 : (mb + 1) * 128],
                            wo_sb[:, kc, nt * 512 : (nt + 1) * 512],
                            start=(kc == 0),
                            stop=(kc == 3),
                        )
                    if nt == 0:
                        nc.scalar.activation(ot[:, 0:512], ps, Copy)
                    else:
                        nc.vector.tensor_copy(ot[:, 512:1024], ps)
                nc.gpsimd.dma_start(out=out[mb * 128 : (mb + 1) * 128, :], in_=ot)


_NC_CACHE = []


def _build():
    if _NC_CACHE:
        return _NC_CACHE[0]
    nc = bacc.Bacc(None, target_bir_lowering=False, debug=False)
    x1 = nc.dram_tensor("x1", [64, 16, S], FP8, kind="ExternalInput")
    x2f = nc.dram_tensor("x2f", [64, 16, S], FP8, kind="ExternalInput")
    x2b = nc.dram_tensor("x2b", [D_IN, S], BF16, kind="ExternalInput")
    wq8 = nc.dram_tensor("wq8", [64, 2, 4096], FP8, kind="ExternalInput")
    wk8 = nc.dram_tensor("wk8", [64, 2, 4096], FP8, kind="ExternalInput")
    wv = nc.dram_tensor("wv", [D_IN, DS], BF16, kind="ExternalInput")
    wo = nc.dram_tensor("wo", [DS, D_MODEL], BF16, kind="ExternalInput")
    bq = nc.dram_tensor("bq", [DS, 1], F32, kind="ExternalInput")
    bk = nc.dram_tensor("bk", [DS, 1], F32, kind="ExternalInput")
    bv = nc.dram_tensor("bv", [DS, 1], F32, kind="ExternalInput")
    out = nc.dram_tensor("out", [S, D_MODEL], BF16, kind="ExternalOutput")
    with tile.TileContext(nc) as tc:
        _kernel_body(
            nc,
            tc,
            aps=(
                x1[:, :, :],
                x2f[:, :, :],
                x2b[:, :],
                wq8[:, :, :],
                wk8[:, :, :],
                wv[:, :],
                wo[:, :],
                bq[:, :],
                bk[:, :],
                bv[:, :],
                out[:, :],
            ),
        )
    nc.compile()
    _NC_CACHE.append(nc)
    return nc


def _run(inputs, trace=False, **kw):
    import ml_dtypes

    nc = _build()
    F8 = ml_dtypes.float8_e4m3
    BF = ml_dtypes.bfloat16
    f32 = lambda a: np.ascontiguousarray(np.asarray(a, dtype=np.float32))
    perm = _col_perm()
    X1, X2 = (
        np.asarray(inputs["X1"], np.float32),
        np.asarray(inputs["X2"], np.float32),
    )
    Wq, Wk = np.asarray(inputs["Wq"], np.float32), np.asarray(
        inputs["Wk"], np.float32
    )
    Wv, Wo = np.asarray(inputs["Wv"], np.float32), np.asarray(
        inputs["Wo"], np.float32
    )
    bqf, bkf = (
        np.asarray(inputs["bq"], np.float32),
        np.asarray(inputs["bk"], np.float32),
    )

    def to_x8(Xb):  # [S, D_IN] -> [64, 16, S] fp8, d = 128*kc + 64*j + p
        a = Xb.T.reshape(NKC, 2, 64, S).transpose(2, 1, 0, 3)
        return np.ascontiguousarray(a.reshape(64, 16, S)).astype(F8)

    def to_w8(Ws):  # [D_IN, DS] (col-perm'd) -> [64, 2, 4096]
        a = Ws.reshape(NKC, 2, 64, 2, 256).transpose(2, 3, 1, 0, 4)
        return np.ascontiguousarray(a.reshape(64, 2, 4096)).astype(F8)

    in_maps = []
    for c in range(8):
        b, hf = c // 2, c % 2
        sl = slice(hf * DS, (hf + 1) * DS)
        wq_s, wk_s = Wq[:, sl][:, perm], Wk[:, sl][:, perm]
        in_maps.append(
            {
                "x1": to_x8(X1[b]),
                "x2f": to_x8(X2[b]),
                "x2b": np.ascontiguousarray(X2[b].T).astype(BF),
                "wq8": to_w8(wq_s),
                "wk8": to_w8(wk_s),
                "wv": np.ascontiguousarray(Wv[:, sl]).astype(BF),
                "wo": np.ascontiguousarray(Wo[sl, :]).astype(BF),
                "bq": np.ascontiguousarray(bqf[sl][perm]).reshape(DS, 1),
                "bk": np.ascontiguousarray(bkf[sl][perm]).reshape(DS, 1),
                "bv": f32(inputs["bv"])[sl].reshape(DS, 1),
            }
        )
    res = run_bass_kernel_spmd(nc, in_maps, list(range(8)), trace=trace, **kw)
    parts = [np.asarray(res.results[c]["out"], np.float32) for c in range(8)]
    bo = f32(inputs["bo"])
    full = np.stack(
        [parts[2 * b] + parts[2 * b + 1] + bo[None, :] for b in range(B)]
    )
    return full.astype(np.float32), res


def kernel(**inputs):
    out, _ = _run(inputs, trace=False)
    return out


# revision 25
# speedup vs baseline: 1.0566x; 1.0566x over previous
"""MultiHeadAttention Trainium2 kernel — fp8 DoubleRow Q/K projections +
fp8 DoubleRow scores + dual-engine (ACT exact / DVE fast-exp) softmax +
bf16 attention/output path.

Core c: batch b=c//2, heads [(c%2)*8, (c%2)*8+8) (512-wide D_MODEL slice).
Host sums the two partial output projections per batch and adds bo.

Cost-model-driven design (TimelineSim charges matmuls out_free_size c/row;
bf16 = 1 c/row, fp8 DoubleRow = 0.5 c/row contracting 2x128 rows per
instruction; ACT = 0.83 ns/elem, DVE = 1.04 ns/elem from f32 psum):
  - Q/K projections fp8 DoubleRow (x1/x2/wq/wk host-quantized to fp8 in
    [64, 2pair, kc, .] layout; wq/wk columns host-permuted so psum
    partitions land in the scrambled (hmod4*32 + dmod32) layout that
    scores DoubleRow wants); the bias-add+quantize to fp8 qt8/kt8 runs on
    ACT (Identity+bias) or DVE (tensor_scalar) whichever is less loaded.
  - V projection bf16 (from a separate bf16 copy of x2): fp8 V error
    (~2.5%/elem) passes through attention averaging undamped and would
    blow the 2e-2 gate; bf16 V is ~0.2%.
  - scores per head = ONE DoubleRow matmul [32,2,128]x[32,2,512] ->
    psum [128 k, 512 q].
  - exp: split between ACT (exact activation, ~1.04us/tick) and DVE
    (Schraudolph fast-exp: i16 = round(score*16*log2e + 16248.5) written
    into the bf16 ex tile via .bitcast(int16) = piecewise-linear exp with
    ~1.8% rms / 4.2% max per-element error; quota-capped so total output
    error stays well under the 2e-2 gate). Both produce bf16 ex tiles;
    attn@V and den read them identically.
  - attn@V bf16 as before; denominators via 1-column ones matmuls.
  - normalize: one DVE reciprocal [128,8] per pair + ONE broadcast
    tensor_tensor (outp * rden with a stride-0 ap) -> ao bf16; xbar
    DMA-transpose -> aot.
  - output projection bf16; psum halves drained by ACT-Copy or DVE-copy
    (least-loaded), DMA'd to HBM from the gpsimd (Pool) queue -- Pool is
    otherwise idle and its SWDGE descriptor generation is free ACT/DVE
    time.
Emission is one flat software-pipelined stream over 256 (qb,pair,kc)
ticks as in the previous version: per tick: exp(t) on the engine with the
earlier modeled finish | loads | one EDF-scheduled heavy PE fill | deferred
attn@V from a bounded backlog (age-released, v-chunk-gated) | scores(t+2)
last. PSUM: scores 2x2 banks + outp 2x1 + den 1 + fill scratch 1 = 8.
"""

import os as _os
import sys

sys.path.insert(0, "/opt/trn_rl_repo")

from collections import defaultdict
from contextlib import ExitStack

import numpy as np
import concourse.bass as bass
import concourse.tile as tile
from concourse import bacc, mybir
from concourse.bass_utils import run_bass_kernel_spmd

B, S, D_IN, D_MODEL, H = 4, 2048, 1024, 1024, 16
DH = 64
HPC = 8
DS = 512
F32 = mybir.dt.float32
BF16 = mybir.dt.bfloat16
FP8 = mybir.dt.float8e4
I16 = mybir.dt.int16
DRow = mybir.MatmulPerfMode.DoubleRow
Exp = mybir.ActivationFunctionType.Exp
Ident = mybir.ActivationFunctionType.Identity
Copy = mybir.ActivationFunctionType.Copy
Mult = mybir.AluOpType.mult
Add = mybir.AluOpType.add

NKC = D_IN // 128  # 8
NSC = S // 128  # 16
QB = 512
SCALE = 1.0 / np.sqrt(DH)
# Schraudolph fast-exp on DVE: bf16 bits of e^(score*SCALE) ~=
# round(score * 128*SCALE*log2(e) + 127*128 - 7.5); -7.5 centers the
# piecewise-linear sawtooth (1.8% rms / 4.2% max, measured on HW).
AEXP = 16.0 * float(np.log2(np.e))
BEXP = 16256.0 - 7.5
DVE_EXP_MAX = int(_os.environ.get("K_DVEMAX", 96))  # max fast-exp ticks

D_MIN = 2  # min backlog: released attn@V must be stale so it never gates PE
AGE = int(_os.environ.get("K_AGE", 12))  # attn@V runs AGE ticks behind exp
MAX_BACKLOG = int(_os.environ.get("K_MAXBL", 16))
QLAG = 2  # producer-to-consumer deferral ticks (psum scratch quantize/drain)
TLAG_ENV = None
NLAG = 2  # attn@V-end to normalize
TLAG = int(_os.environ.get("K_TLAG", 4))  # normalize to xbar transposes
EX_BUFS = 18


def _col_perm():
    """Permuted D-column order for wq/wk so proj psum partitions match the
    scores-DoubleRow layout: chunk c=(hslot,dj), partition p ->
    col = 64*(p//32 + 4*hslot) + 32*dj + p%32."""
    perm = np.empty(DS, np.int64)
    for c in range(4):
        hslot, dj = c // 2, c % 2
        for p in range(128):
            perm[c * 128 + p] = 64 * (p // 32 + 4 * hslot) + 32 * dj + p % 32
    return perm


def _kernel_body(nc, tc, aps):
    x1, x2f, x2b, wq8, wk8, wv, wo, bq, bk, bv, out = aps

    with ExitStack() as ctx:
        pers = ctx.enter_context(tc.tile_pool(name="pers", bufs=1))

        wq8_sb = pers.tile([64, 2, 2, NKC, 256], FP8)  # [p, half, j, kc, ci]
        wk8_sb = pers.tile([64, 2, 2, NKC, 256], FP8)
        wv_sb = pers.tile([128, NKC, DS], BF16)
        wo_sb = pers.tile([128, 4, D_MODEL], BF16)
        x2f_sb = pers.tile([64, 2, NKC, S], FP8)  # [p, j, kc, t]
        x2b_sb = pers.tile([128, NKC, S], BF16)
        qt8 = pers.tile([128, 2, 2, S], FP8)  # [p, hslot, dj, q]
        kt8 = pers.tile([128, 2, 2, S], FP8)
        v_sb = pers.tile([128, NSC, HPC, DH], BF16)
        aot = pers.tile([128, 4, S], BF16)
        bq_sb = pers.tile([128, 4], F32)
        bk_sb = pers.tile([128, 4], F32)
        bv_bc = pers.tile([128, DS], F32)
        ones = pers.tile([128, 1], BF16)

        px1 = ctx.enter_context(tc.tile_pool(name="px1", bufs=2))
        x1ts = {}

        nc.gpsimd.memset(ones[:, :], 1.0)
        nc.gpsimd.dma_start(
            out=bq_sb, in_=bq.rearrange("(c p) o -> p (c o)", p=128)
        )
        nc.gpsimd.dma_start(
            out=bk_sb, in_=bk.rearrange("(c p) o -> p (c o)", p=128)
        )
        nc.gpsimd.dma_start(
            out=bv_bc, in_=bv.rearrange("s o -> o s").to_broadcast([128, DS])
        )

        # ---- engine-clock model (for exp/quantize/drain placement) ----
        eng_clk = {"a": 0.0, "d": 0.0}
        C_EA, C_ED = 1040.0, 1290.0  # exp per tick
        C_QA, C_QD = 600.0, 670.0  # qk quantize [128,512]
        C_VB = 670.0  # v bias (DVE only)
        C_NRM = 880.0  # normalize per pair (DVE only)
        C_DA, C_DD = 600.0, 670.0  # oproj drain half
        dve_exp_used = [0]

        def pick_eng(ca, cd):
            if eng_clk["a"] + ca <= eng_clk["d"] + cd:
                eng_clk["a"] += ca
                return "a"
            eng_clk["d"] += cd
            return "d"

        def ld_x1(sq, eng):
            t = px1.tile([64, 2, NKC, QB], FP8, tag="x8", name=f"x1_{sq}")
            eng.dma_start(
                out=t,
                in_=x1[:, :, sq * QB : (sq + 1) * QB].rearrange(
                    "p (j c) s -> p j c s", j=2
                ),
            )
            x1ts[sq] = t

        def ld_x2f(sq, eng):
            eng.dma_start(
                out=x2f_sb[:, :, :, sq * QB : (sq + 1) * QB],
                in_=x2f[:, :, sq * QB : (sq + 1) * QB].rearrange(
                    "p (j c) s -> p j c s", j=2
                ),
            )

        def ld_x2b(sq, eng):
            eng.dma_start(
                out=x2b_sb[:, :, sq * QB : (sq + 1) * QB],
                in_=x2b.rearrange("(c p) s -> p c s", p=128)[
                    :, :, sq * QB : (sq + 1) * QB
                ],
            )

        # fp8 DoubleRow projection chunk c of quarter sq -> fp8 qt8/kt8
        # (columns host-permuted so psum partitions land in the
        # scores-DoubleRow layout); bias-add+fp8-quantize on the less
        # loaded of ACT/DVE.
        # Cross-engine consumers (quantize / bias / drain / normalize /
        # transpose) are NOT emitted right after their producer matmuls:
        # engines execute in order, so a dep-waiting op at the head of
        # ACT/DVE would stall the exp stream behind it. Instead they are
        # queued here and emitted 1+ ticks later, when the producer is
        # (nearly) done. Safe with single-buffer psum scratch because
        # deferred thunks run BEFORE the tick's fill job allocates.
        deferred = defaultdict(list)
        cur_tick = [0]

        def qk_chunk(psp, which, sq, c):
            w_sb, dst, b_sb = (
                (wq8_sb, qt8, bq_sb) if which == 1 else (wk8_sb, kt8, bk_sb)
            )
            xt = x1ts[sq] if which == 1 else x2f_sb
            h, c2 = c // 2, c % 2
            ps = psp.tile([128, QB], F32, tag="pp", name="qkp")
            for kc in range(NKC):
                if which == 1:
                    mov = xt[:, :, kc, :]
                else:
                    mov = xt[:, :, kc, sq * QB : (sq + 1) * QB]
                nc.tensor.matmul(
                    ps,
                    w_sb[:, h, :, kc, c2 * 128 : (c2 + 1) * 128],
                    mov,
                    start=(kc == 0),
                    stop=(kc == NKC - 1),
                    perf_mode=DRow,
                )
            dsts = dst[:, c // 2, c % 2, sq * QB : (sq + 1) * QB]

            def quant():
                if pick_eng(C_QA, C_QD) == "a":
                    nc.scalar.activation(
                        dsts, ps, Ident, bias=b_sb[:, c : c + 1]
                    )
                else:
                    nc.vector.tensor_scalar_add(dsts, ps, b_sb[:, c : c + 1])

            return quant

        def v_chunk(psp, sq, sc):
            ps = psp.tile([128, QB], F32, tag="pp", name="vp")
            for kc in range(NKC):
                nc.tensor.matmul(
                    ps,
                    x2b_sb[:, kc, sq * QB + sc * 128 : sq * QB + (sc + 1) * 128],
                    wv_sb[:, kc, :],
                    start=(kc == 0),
                    stop=(kc == NKC - 1),
                )

            def vbias():
                eng_clk["d"] += C_VB
                nc.vector.tensor_add(
                    v_sb[:, sq * 4 + sc, :, :],
                    ps.rearrange("p (h d) -> p h d", h=HPC),
                    bv_bc.rearrange("p (h d) -> p h d", h=HPC),
                )

            return vbias

        # ---- load streams ----
        # DMA transfers serialize on the modeled (exclusive) DMA device, so
        # order IS the schedule. Scalar carries only the loads the first
        # scores need; everything else on sync. ACT/DVE never issue DMAs
        # (their 667ns seq dispatch would stall the exp stream).
        dma_ready = {}
        _dma_clk = [2000.0]

        def dma_ns(total_bytes, elem):
            lat = 2.0 if elem < 512 else 1.0
            return total_bytes / elem / 16.0 * max(elem * lat / 22.5, 7.0)

        def _track(name, total_bytes, elem, emit):
            emit()
            _dma_clk[0] += dma_ns(total_bytes, elem) + 800.0
            dma_ready[name] = _dma_clk[0]

        KB = 1024
        _track("wk8h", 256 * KB, 4096,
               lambda: nc.scalar.dma_start(out=wk8_sb[:, 0], in_=wk8[:, 0, :].rearrange("p (j c i) -> p j c i", j=2, c=NKC)))
        _track("x2f0", 512 * KB, 512, lambda: ld_x2f(0, nc.scalar))
        _track("wq8h", 256 * KB, 4096,
               lambda: nc.sync.dma_start(out=wq8_sb[:, 0], in_=wq8[:, 0, :].rearrange("p (j c i) -> p j c i", j=2, c=NKC)))
        _track("x1q0", 512 * KB, 512, lambda: ld_x1(0, nc.sync))
        _track("x2f1", 512 * KB, 512, lambda: ld_x2f(1, nc.sync))
        _track("wv", 1024 * KB, 1024,
               lambda: nc.sync.dma_start(out=wv_sb, in_=wv.rearrange("(c p) o -> p c o", p=128)))
        _track("x2f2", 512 * KB, 512, lambda: ld_x2f(2, nc.sync))
        _track("x2b0", 1024 * KB, 1024, lambda: ld_x2b(0, nc.sync))
        _track("x2f3", 512 * KB, 512, lambda: ld_x2f(3, nc.sync))
        _track("x2b1", 1024 * KB, 1024, lambda: ld_x2b(1, nc.sync))
        _track("x2b2", 1024 * KB, 1024, lambda: ld_x2b(2, nc.sync))

        # startup projection chunks in their own psum pool. Dummy matmuls
        # first ramp the PE clock out of its cold p-state.
        warm = pers.tile([128, 512], BF16)
        nc.gpsimd.memset(warm[:, :], 0.0)
        with tc.tile_pool(name="psA", bufs=4, space="PSUM") as psA:
            wps = psA.tile([128, 512], F32, tag="pp", name="warmp")
            for i in range(8):
                nc.tensor.matmul(
                    wps[0:1, :],
                    ones[:, 0:1],
                    warm[:, :],
                    start=(i == 0),
                    stop=(i == 7),
                )
            qk_chunk(psA, 2, 0, 0)()
            qk_chunk(psA, 2, 0, 1)()
            qk_chunk(psA, 1, 0, 0)()
            qk_chunk(psA, 1, 0, 1)()

        # ---- attention pools ----
        attn_ctx = ctx.enter_context(ExitStack())
        pso = attn_ctx.enter_context(tc.tile_pool(name="pso", bufs=1, space="PSUM"))
        pdn = attn_ctx.enter_context(tc.tile_pool(name="pdn", bufs=1, space="PSUM"))
        pex = attn_ctx.enter_context(tc.tile_pool(name="pex", bufs=EX_BUFS))
        pao = attn_ctx.enter_context(tc.tile_pool(name="pao", bufs=2))
        prd = attn_ctx.enter_context(tc.tile_pool(name="prd", bufs=2))
        pot = attn_ctx.enter_context(tc.tile_pool(name="pot", bufs=2))
        # created last so they can close first (LIFO), freeing 6 psum banks
        # for the tail oproj pipeline
        sc_ctx = ExitStack()
        psc = sc_ctx.enter_context(tc.tile_pool(name="psc", bufs=2, space="PSUM"))
        psp = sc_ctx.enter_context(tc.tile_pool(name="psp", bufs=2, space="PSUM"))

        ot_tiles = {}

        def oproj_half(mb, nt):
            if nt == 0:
                ot_tiles[mb] = pot.tile(
                    [128, D_MODEL], BF16, tag="ot", name="oti"
                )
            ot = ot_tiles[mb]
            ps = psp.tile([128, 512], F32, tag="pp", name="opp")
            for kc in range(4):
                nc.tensor.matmul(
                    ps,
                    aot[:, kc, mb * 128 : (mb + 1) * 128],
                    wo_sb[:, kc, nt * 512 : (nt + 1) * 512],
                    start=(kc == 0),
                    stop=(kc == 3),
                )

            def drain():
                if pick_eng(C_DA, C_DD) == "a":
                    nc.scalar.activation(
                        ot[:, nt * 512 : (nt + 1) * 512], ps, Copy
                    )
                else:
                    nc.vector.tensor_copy(ot[:, nt * 512 : (nt + 1) * 512], ps)
                if nt == 1:
                    nc.gpsimd.dma_start(
                        out=out[mb * 128 : (mb + 1) * 128, :], in_=ot
                    )

            return drain

        # ---- filler schedule: tick -> thunks ----
        # chunk c=(hslot,dj) of a quarter serves pairs 2*(c//2), 2*(c//2)+1.
        CQK, CV, COPH = 860, 1707, 860
        T0 = float(_os.environ.get("K_T0", 12000.0))
        TICK = float(_os.environ.get("K_TICK", 960.0))

        def r2t(ns):
            return max(0, int((ns - T0) / TICK) + 1)

        # jobs: (deadline_tick, ready_tick, pe_cost, thunk, tag). One heavy
        # job per tick, earliest-deadline-first among ready jobs. qt/kt
        # chunks MUST land by their deadline (scores would otherwise read
        # uninitialized sbuf = a real race); v jobs may slip (attn@V release
        # is gated on the v chunk being emitted, the backlog absorbs it).
        jobs = []
        for s in range(1, 4):
            for c in range(2):
                jobs.append(
                    (4 * s - 4 + c, r2t(dma_ready[f"x2f{s}"]), CQK,
                     lambda s=s, c=c: qk_chunk(psp, 2, s, c), None, 4 * s)
                )
        # pair 2 (tick 32+4s) reads BOTH dj chunks (c2 and c3) of hslot 1
        # c2/c3 need the half-1 weight loads dispatched at fill ticks 3/4:
        # ready >= 6 also orders the emission after those dma_starts.
        for s in range(4):
            rd = r2t(dma_ready[f"x2f{s}"])
            jobs.append(
                (28 + 4 * s, max(rd, 6), CQK,
                 lambda s=s: qk_chunk(psp, 2, s, 2), None, 32 + 4 * s)
            )
            jobs.append(
                (29 + 4 * s, max(rd, 6), CQK,
                 lambda s=s: qk_chunk(psp, 2, s, 3), None, 32 + 4 * s)
            )
        for c in (2, 3):
            jobs.append(
                (26 + c, max(r2t(dma_ready["x1q0"]), 6), CQK,
                 lambda c=c: qk_chunk(psp, 1, 0, c), None, 32)
            )
        for s in range(4):
            rv = r2t(max(dma_ready.get(f"x2b{s}", 0.0), dma_ready["wv"])) \
                if s < 3 else 28
            for sc in range(4):
                dl = max(4 * s + sc + AGE - 1, rv)
                jobs.append(
                    (dl, rv, CV,
                     lambda s=s, sc=sc: v_chunk(psp, s, sc), ("v", 4 * s + sc),
                     None)
                )
        for sq in range(1, 4):
            for c in range(4):
                jobs.append(
                    (64 * sq - 4 + c if c < 2 else 64 * sq + 26 + c,
                     64 * (sq - 1) + 6, CQK,
                     lambda sq=sq, c=c: qk_chunk(psp, 1, sq, c), None,
                     64 * sq if c < 2 else 64 * sq + 32)
                )
        # oproj halves: the source qb's last transpose is emitted when its
        # final attn@V leaves the backlog (qb end + AGE)
        # aot for block qb is complete once pair3's transposes are emitted:
        # release(qb,3,15) ~ 64qb+63+AGE, + NLAG (norm) + TLAG (tps) + margin
        for mb in range(12):
            for nt in range(2):
                jobs.append(
                    (1000 + 2 * mb + nt,
                     64 * (mb // 4 + 1) + AGE + TLAG + 4, COPH,
                     lambda mb=mb, nt=nt: oproj_half(mb, nt), None, None)
                )
        jobs.sort(key=lambda j: (j[0], j[1]))
        # Precompute the EDF tick assignment; qt/kt jobs must land by their
        # deadline, v jobs gate attn@V release below.
        v_done_tick = {}
        _pending = list(range(len(jobs)))
        assigned = {}
        for t in range(256):
            pick = None
            for idx in _pending:
                if jobs[idx][0] <= t or jobs[idx][1] <= t:
                    pick = idx
                    break
            if pick is not None:
                assigned[t] = pick
                _pending.remove(pick)
                tag = jobs[pick][4]
                if tag and tag[0] == "v":
                    v_done_tick[tag[1]] = t
                rt = jobs[pick][5]
                if rt is not None:
                    assert t <= rt - 3, (t, rt)
        assert not _pending, f"{len(_pending)} jobs unassigned"
        assert len(v_done_tick) == 16
        # zero-cost emissions (loads) at fixed ticks
        fill = defaultdict(list)
        fill[2].append(lambda: ld_x2b(3, nc.sync))
        fill[3].append(
            lambda: nc.sync.dma_start(out=wk8_sb[:, 1], in_=wk8[:, 1, :].rearrange("p (j c i) -> p j c i", j=2, c=NKC))
        )
        fill[4].append(
            lambda: nc.sync.dma_start(out=wq8_sb[:, 1], in_=wq8[:, 1, :].rearrange("p (j c i) -> p j c i", j=2, c=NKC))
        )
        fill[5].append(lambda: ld_x1(1, nc.sync))
        for sq in range(2, 4):
            fill[64 * (sq - 1) + 2].append(lambda sq=sq: ld_x1(sq, nc.sync))
        fill[20].append(
            lambda: nc.sync.dma_start(
                out=wo_sb, in_=wo.rearrange("(c p) o -> p c o", p=128)
            )
        )

        # ---- flat pipelined attention stream ----
        TICKS = [
            (qb, pair, kc)
            for qb in range(4)
            for pair in range(4)
            for kc in range(NSC)
        ]
        sc_tiles = {}
        ex_tiles = {}
        state = {}

        def emit_sc(t):
            qb, pair, kc = TICKS[t]
            q0 = qb * QB
            scp = psc.tile([128, 2, QB], F32, tag="sc", name="scp")
            for par in range(2):
                h = 2 * pair + par
                hb = 32 * (h % 4)
                nc.tensor.matmul(
                    scp[:, par, :],
                    kt8[hb : hb + 32, h // 4, :, kc * 128 : (kc + 1) * 128],
                    qt8[hb : hb + 32, h // 4, :, q0 : q0 + QB],
                    start=True,
                    stop=True,
                    perf_mode=DRow,
                    tile_position=(hb, 0),
                )
            sc_tiles[t] = scp

        def emit_exp(t, ready_ns):
            expt = pex.tile([128, 2, QB], BF16, tag="ex", name="ext")
            scp = sc_tiles.pop(t)
            fa = max(eng_clk["a"], ready_ns) + C_EA
            fd = max(eng_clk["d"], ready_ns) + C_ED
            if fd < fa and dve_exp_used[0] < DVE_EXP_MAX:
                dve_exp_used[0] += 1
                eng_clk["d"] = fd
                nc.vector.tensor_scalar(
                    expt[:, :, :].bitcast(I16), scp, AEXP, BEXP, Mult, Add
                )
            else:
                eng_clk["a"] = fa
                nc.scalar.activation(expt, scp, Exp, scale=float(SCALE))
            ex_tiles[t] = expt

        def emit_av(t):
            qb, pair, kc = TICKS[t]
            expt = ex_tiles.pop(t)
            if kc == 0:
                state["outp"] = pso.tile(
                    [128, 2, 4, DH], F32, tag="acc", name="outp"
                )
                if pair == 0:
                    state["den"] = pdn.tile([128, 32], F32, tag="dn", name="den")
                    state["rden"] = prd.tile([128, 32], F32, tag="rd", name="rden")
            outp = state["outp"]
            den = state["den"]
            for par in range(2):
                h = 2 * pair + par
                for qc in range(4):
                    exs = expt[:, par, qc * 128 : (qc + 1) * 128]
                    first = kc == 0 and par == 0 and qc == 0
                    last = kc == NSC - 1 and par == 1 and qc == 3
                    nc.tensor.matmul(
                        outp[:, par, qc, :],
                        exs,
                        v_sb[:, kc, h, :],
                        start=first,
                        stop=last,
                    )
                    di = pair * 8 + par * 4 + qc
                    nc.tensor.matmul(
                        den[:, di : di + 1],
                        exs,
                        ones[:, :],
                        start=first,
                        stop=last,
                    )
            if kc == NSC - 1:
                q0 = qb * QB
                rden = state["rden"]
                # reciprocal inline: it must execute before the next pair's
                # den accumulation group opens in the same psum tensor
                rsl = rden[:, pair * 8 : (pair + 1) * 8]
                nc.vector.reciprocal(rsl, den[:, pair * 8 : (pair + 1) * 8])
                ao = pao.tile([128, 4, 2, DH], BF16, tag="ao", name="ao")
                nc.vector.tensor_tensor(
                    out=ao[:, :, :, :],
                    in0=outp.rearrange("p a b d -> p b a d"),
                    in1=rsl.rearrange("p (a b) -> p b a", a=2).to_broadcast(
                        [128, 4, 2, DH]
                    ),
                    op=Mult,
                )
                eng_clk["d"] += C_NRM

                def tps():
                    for qc in range(4):
                        nc.sync.dma_start_transpose(
                            aot[:, pair, q0 + qc * 128 : q0 + (qc + 1) * 128],
                            ao[:, qc, :, :],
                        )

                deferred[cur_tick[0] + TLAG].append(tps)

        # Greedy emission: track modeled PE/ACT/DVE clocks; defer attn@V
        # work (bounded backlog) and drain it age-based so the ex pool never
        # starves the exp stream. Never release an attn@V whose v chunk
        # hasn't been emitted yet (emission order defines dependency order).
        C_SC, C_AV = 213.0, 220.0
        pe_t = 5500.0  # first matmul lands after the startup DMA chain
        eng_clk["a"] = eng_clk["d"] = 7000.0
        sc_done = {}
        backlog = []

        emit_sc(0)
        sc_done[0] = pe_t = pe_t + C_SC
        emit_sc(1)
        sc_done[1] = pe_t = pe_t + C_SC
        for t in range(256):
            cur_tick[0] = t
            # deferred producers-consumers first: their inputs are ~done, and
            # queue position ahead of exp(t) lets vbias/quant unblock the
            # attn@V -> ex-pool chain instead of sitting behind a 1.2us exp
            for th in deferred.pop(t, ()):
                th()
            emit_exp(t, sc_done[t] + 100.0)
            backlog.append(t)
            for f in fill[t]:
                f()
            if t in assigned:
                dl, rd, cost, th, tag, rt = jobs[assigned[t]]
                post = th()
                pe_t += cost
                if post is not None:
                    # the quantize must be emitted before the scores reading
                    # its output region (emitted at tick rt-2, sc phase)
                    lag = QLAG if rt is None else max(1, min(QLAG, rt - 2 - t))
                    deferred[t + lag].append(post)
            # attn@V after the job: these small matmuls overlap the job's
            # psum-drain latency so back-to-back fills don't bubble PE.
            while backlog and (
                len(backlog) > MAX_BACKLOG
                or (len(backlog) > D_MIN and t - backlog[0] >= AGE)
            ):
                qbu, pairu, kcu = TICKS[backlog[0]]
                if qbu == 0 and v_done_tick[kcu] + QLAG > t:
                    break
                emit_av(backlog.pop(0))
                pe_t += C_AV
            # exp-gated score matmul last, so jobs/attn@V never sit behind
            # the gate in the PE queue
            if t + 2 < 256:
                emit_sc(t + 2)
                pe_t += C_SC
                sc_done[t + 2] = pe_t
        # psc/psp are done once the in-loop deferred quantizes/drains flush;
        # release their 6 banks for the tail oproj pipeline
        sc_ctx.close()

        # ---- tail: drain backlog while accumulating the last oproj blocks'
        # first three contraction steps (aot pairs 0-2 are long since
        # transposed); pair 3's chunk + drains follow the final transposes.
        psD_ctx = ExitStack()
        psD = psD_ctx.enter_context(tc.tile_pool(name="psD", bufs=4, space="PSUM"))
        tail_ps = {}
        tail_ot = {}

        def tail_phaseA(mb, nt):
            if nt == 0:
                tail_ot[mb] = pot.tile(
                    [128, D_MODEL], BF16, tag="ot", name="otd"
                )
            ps = psD.tile([128, 512], F32, tag="pf", name="opd")
            tail_ps[(mb, nt)] = ps
            for kc in range(3):
                nc.tensor.matmul(
                    ps,
                    aot[:, kc, mb * 128 : (mb + 1) * 128],
                    wo_sb[:, kc, nt * 512 : (nt + 1) * 512],
                    start=(kc == 0),
                    stop=False,
                )

        def tail_finish(mb, nt):
            ps = tail_ps.pop((mb, nt))
            ot = tail_ot[mb]
            nc.tensor.matmul(
                ps,
                aot[:, 3, mb * 128 : (mb + 1) * 128],
                wo_sb[:, 3, nt * 512 : (nt + 1) * 512],
                start=False,
                stop=True,
            )
            if nt == 0:
                nc.scalar.activation(ot[:, 0:512], ps, Copy)
            else:
                nc.vector.tensor_copy(ot[:, 512:1024], ps)
                nc.sync.dma_start(out=out[mb * 128 : (mb + 1) * 128, :], in_=ot)

        for u in backlog:
            cur_tick[0] += 1
            emit_av(u)
            for th in deferred.pop(cur_tick[0], ()):
                th()
        # phase-A after the backlog (PE would idle during the final
        # norm/transposes otherwise; before it, it would delay them)
        for mb, nt in ((12, 0), (12, 1), (13, 0), (13, 1)):
            tail_phaseA(mb, nt)
        while deferred:
            t = min(deferred)
            cur_tick[0] = max(cur_tick[0], t)
            for th in deferred.pop(t):
                th()
        # final transposes are emitted by now; finish 12/13, pipeline 14/15
        tail_finish(12, 0)
        tail_finish(12, 1)
        tail_phaseA(14, 0)
        tail_phaseA(14, 1)
        tail_finish(13, 0)
        tail_finish(13, 1)
        tail_phaseA(15, 0)
        tail_phaseA(15, 1)
        tail_finish(14, 0)
        tail_finish(14, 1)
        tail_finish(15, 0)
        tail_finish(15, 1)
        psD_ctx.close()
        attn_ctx.close()


_NC_CACHE = []


def _build():
    if _NC_CACHE:
        return _NC_CACHE[0]
    nc = bacc.Bacc(None, target_bir_lowering=False, debug=False)
    x1 = nc.dram_tensor("x1", [64, 16, S], FP8, kind="ExternalInput")
    x2f = nc.dram_tensor("x2f", [64, 16, S], FP8, kind="ExternalInput")
    x2b = nc.dram_tensor("x2b", [D_IN, S], BF16, kind="ExternalInput")
    wq8 = nc.dram_tensor("wq8", [64, 2, 4096], FP8, kind="ExternalInput")
    wk8 = nc.dram_tensor("wk8", [64, 2, 4096], FP8, kind="ExternalInput")
    wv = nc.dram_tensor("wv", [D_IN, DS], BF16, kind="ExternalInput")
    wo = nc.dram_tensor("wo", [DS, D_MODEL], BF16, kind="ExternalInput")
    bq = nc.dram_tensor("bq", [DS, 1], F32, kind="ExternalInput")
    bk = nc.dram_tensor("bk", [DS, 1], F32, kind="ExternalInput")
    bv = nc.dram_tensor("bv", [DS, 1], F32, kind="ExternalInput")
    out = nc.dram_tensor("out", [S, D_MODEL], BF16, kind="ExternalOutput")
    with tile.TileContext(nc) as tc:
        _kernel_body(
            nc,
            tc,
            aps=(
                x1[:, :, :],
                x2f[:, :, :],
                x2b[:, :],
                wq8[:, :, :],
                wk8[:, :, :],
                wv[:, :],
                wo[:, :],
                bq[:, :],
                bk[:, :],
                bv[:, :],
                out[:, :],
            ),
        )
    nc.compile()
    _NC_CACHE.append(nc)
    return nc


def _run(inputs, trace=False, **kw):
    import ml_dtypes

    nc = _build()
    F8 = ml_dtypes.float8_e4m3
    BF = ml_dtypes.bfloat16
    f32 = lambda a: np.ascontiguousarray(np.asarray(a, dtype=np.float32))
    perm = _col_perm()
    X1, X2 = (
        np.asarray(inputs["X1"], np.float32),
        np.asarray(inputs["X2"], np.float32),
    )
    Wq, Wk = np.asarray(inputs["Wq"], np.float32), np.asarray(
        inputs["Wk"], np.float32
    )
    Wv, Wo = np.asarray(inputs["Wv"], np.float32), np.asarray(
        inputs["Wo"], np.float32
    )
    bqf, bkf = (
        np.asarray(inputs["bq"], np.float32),
        np.asarray(inputs["bk"], np.float32),
    )

    def to_x8(Xb):  # [S, D_IN] -> [64, 16, S] fp8, d = 128*kc + 64*j + p
        a = Xb.T.reshape(NKC, 2, 64, S).transpose(2, 1, 0, 3)
        return np.ascontiguousarray(a.reshape(64, 16, S)).astype(F8)

    def to_w8(Ws):  # [D_IN, DS] (col-perm'd) -> [64, 2, 4096]
        a = Ws.reshape(NKC, 2, 64, 2, 256).transpose(2, 3, 1, 0, 4)
        return np.ascontiguousarray(a.reshape(64, 2, 4096)).astype(F8)

    in_maps = []
    for c in range(8):
        b, hf = c // 2, c % 2
        sl = slice(hf * DS, (hf + 1) * DS)
        wq_s, wk_s = Wq[:, sl][:, perm], Wk[:, sl][:, perm]
        in_maps.append(
            {
                "x1": to_x8(X1[b]),
                "x2f": to_x8(X2[b]),
                "x2b": np.ascontiguousarray(X2[b].T).astype(BF),
                "wq8": to_w8(wq_s),
                "wk8": to_w8(wk_s),
                "wv": np.ascontiguousarray(Wv[:, sl]).astype(BF),
                "wo": np.ascontiguousarray(Wo[sl, :]).astype(BF),
                "bq": np.ascontiguousarray(bqf[sl][perm]).reshape(DS, 1),
                "bk": np.ascontiguousarray(bkf[sl][perm]).reshape(DS, 1),
                "bv": f32(inputs["bv"])[sl].reshape(DS, 1),
            }
        )
    res = run_bass_kernel_spmd(nc, in_maps, list(range(8)), trace=trace, **kw)
    parts = [np.asarray(res.results[c]["out"], np.float32) for c in range(8)]
    bo = f32(inputs["bo"])
    full = np.stack(
        [parts[2 * b] + parts[2 * b + 1] + bo[None, :] for b in range(B)]
    )
    return full.astype(np.float32), res


def kernel(**inputs):
    out, _ = _run(inputs, trace=False)
    return out


# revision 35
# speedup vs baseline: 1.0582x; 1.0016x over previous
"""MultiHeadAttention Trainium2 kernel — fp8 DoubleRow Q/K projections +
fp8 DoubleRow scores + dual-engine (ACT exact / DVE fast-exp) softmax +
bf16 attention/output path.

Core c: batch b=c//2, heads [(c%2)*8, (c%2)*8+8) (512-wide D_MODEL slice).
Host sums the two partial output projections per batch and adds bo.

Cost-model-driven design (TimelineSim charges matmuls out_free_size c/row;
bf16 = 1 c/row, fp8 DoubleRow = 0.5 c/row contracting 2x128 rows per
instruction; ACT = 0.83 ns/elem, DVE = 1.04 ns/elem from f32 psum):
  - Q/K projections fp8 DoubleRow (x1/x2/wq/wk host-quantized to fp8 in
    [64, 2pair, kc, .] layout; wq/wk columns host-permuted so proj psum
    partitions land in the scrambled (hmod4*32 + dmod32) layout that
    scores DoubleRow wants); bias-add+fp8-quantize to qt8/kt8 runs on ACT
    (Identity + per-partition bias AP) or DVE, whichever is less loaded.
  - V projection bf16 (from a separate bf16 copy of x2): fp8 V error
    (~2.5%/elem) passes through attention averaging undamped and would
    blow the 2e-2 gate; bf16 V is ~0.2%.
  - scores per head = ONE DoubleRow matmul [32,2,128]x[32,2,512] ->
    psum [128 k, 512 q].
  - exp: split between ACT (exact activation, ~1.04us/tick) and DVE
    (Schraudolph fast-exp: i16 = round(score*16*log2(e) + 16248.5) written
    into the bf16 ex tile via .bitcast(int16) = piecewise-linear exp with
    1.8% rms / 4.2% max per-element error, HW-validated; quota-capped so
    total output error stays well under the 2e-2 gate). Both produce bf16
    ex tiles; attn@V and den read them identically.
  - attn@V bf16; denominators via 1-column ones matmuls into a per-pair
    psum group; normalize = one DVE reciprocal [128,8] + ONE broadcast
    tensor_tensor (outp * rden with a stride-0 in1 ap) -> ao bf16; xbar
    DMA-transpose -> aot; output projection bf16, psum halves drained by
    ACT-Copy or DVE-copy, DMA'd from the gpsimd (Pool) queue.
Scheduling (the wall is a latency chain, not engine capacity):
  - One flat software-pipelined stream over 256 (qb,pair,kc) ticks:
    per tick: deferred producer-consumers | exp(t) on the engine with the
    earlier modeled finish | loads | one EDF-scheduled heavy PE fill |
    age-released attn@V backlog (v-chunk-gated) | scores(t+2) last.
  - Engines execute in order, so psum->sbuf consumers (quantize / v-bias /
    oproj drains) are NOT emitted with their producer matmuls: they are
    deferred 1-2 ticks (psp bufs=2 keeps the fill pipeline 2-deep;
    deferred thunks run before the tick's fill so pool reuse stays
    race-free), otherwise a dep-waiting op at the head of ACT/DVE stalls
    the exp stream (CoreSim + trace verified).
  - qt/kt quantize deadline rule: scores for tick T are emitted at T-2,
    so a chunk first read at T must be quantized by then: job tick <=
    T - 2 - lag, lag adapted per job.
  - xbar transposes deferred TLAG ticks after the normalize so their sem
    waits don't occupy SP.SEQ; the tail overlaps the last oproj blocks'
    first 3 contraction chunks with the backlog drain (psc/psp close
    early to free their 6 psum banks).
PSUM: scores 2x2 banks + fill scratch 2 + outp 1 + den 1 = 8 banks.
"""

import os as _os
import sys

sys.path.insert(0, "/opt/trn_rl_repo")

from collections import defaultdict
from contextlib import ExitStack

import numpy as np
import concourse.bass as bass
import concourse.tile as tile
from concourse import bacc, mybir
from concourse.bass_utils import run_bass_kernel_spmd

B, S, D_IN, D_MODEL, H = 4, 2048, 1024, 1024, 16
DH = 64
HPC = 8
DS = 512
F32 = mybir.dt.float32
BF16 = mybir.dt.bfloat16
FP8 = mybir.dt.float8e4
I16 = mybir.dt.int16
DRow = mybir.MatmulPerfMode.DoubleRow
Exp = mybir.ActivationFunctionType.Exp
Ident = mybir.ActivationFunctionType.Identity
Copy = mybir.ActivationFunctionType.Copy
Mult = mybir.AluOpType.mult
Add = mybir.AluOpType.add

NKC = D_IN // 128  # 8
NSC = S // 128  # 16
QB = 512
SCALE = 1.0 / np.sqrt(DH)
# Schraudolph fast-exp on DVE: bf16 bits of e^(score*SCALE) ~=
# round(score * 128*SCALE*log2(e) + 127*128 - 7.5); -7.5 centers the
# piecewise-linear sawtooth (1.8% rms / 4.2% max, measured on HW).
AEXP = 16.0 * float(np.log2(np.e))
BEXP = 16256.0 - 7.5
DVE_EXP_MAX = int(_os.environ.get("K_DVEMAX", 96))  # max fast-exp ticks

D_MIN = 2  # min backlog: released attn@V must be stale so it never gates PE
AGE = int(_os.environ.get("K_AGE", 12))  # attn@V runs AGE ticks behind exp
MAX_BACKLOG = int(_os.environ.get("K_MAXBL", 16))
QLAG = 2  # producer-to-consumer deferral ticks (psum scratch quantize/drain)
TLAG_ENV = None
NLAG = 2  # attn@V-end to normalize
TLAG = int(_os.environ.get("K_TLAG", 4))  # normalize to xbar transposes
EX_BUFS = 18


def _col_perm():
    """Permuted D-column order for wq/wk so proj psum partitions match the
    scores-DoubleRow layout: chunk c=(hslot,dj), partition p ->
    col = 64*(p//32 + 4*hslot) + 32*dj + p%32."""
    perm = np.empty(DS, np.int64)
    for c in range(4):
        hslot, dj = c // 2, c % 2
        for p in range(128):
            perm[c * 128 + p] = 64 * (p // 32 + 4 * hslot) + 32 * dj + p % 32
    return perm


def _kernel_body(nc, tc, aps):
    x1, x2f, x2b, wq8, wk8, wv, wo, bq, bk, bv, out = aps

    with ExitStack() as ctx:
        pers = ctx.enter_context(tc.tile_pool(name="pers", bufs=1))

        wq8_sb = pers.tile([64, 2, 2, NKC, 256], FP8)  # [p, half, j, kc, ci]
        wk8_sb = pers.tile([64, 2, 2, NKC, 256], FP8)
        wv_sb = pers.tile([128, NKC, DS], BF16)
        wo_sb = pers.tile([128, 4, D_MODEL], BF16)
        x2f_sb = pers.tile([64, 2, NKC, S], FP8)  # [p, j, kc, t]
        x2b_sb = pers.tile([128, NKC, S], BF16)
        qt8 = pers.tile([128, 2, 2, S], FP8)  # [p, hslot, dj, q]
        kt8 = pers.tile([128, 2, 2, S], FP8)
        v_sb = pers.tile([128, NSC, HPC, DH], BF16)
        aot = pers.tile([128, 4, S], BF16)
        bq_sb = pers.tile([128, 4], F32)
        bk_sb = pers.tile([128, 4], F32)
        bv_bc = pers.tile([128, DS], F32)
        ones = pers.tile([128, 1], BF16)

        px1 = ctx.enter_context(tc.tile_pool(name="px1", bufs=2))
        x1ts = {}

        nc.gpsimd.memset(ones[:, :], 1.0)
        nc.gpsimd.dma_start(
            out=bq_sb, in_=bq.rearrange("(c p) o -> p (c o)", p=128)
        )
        nc.gpsimd.dma_start(
            out=bk_sb, in_=bk.rearrange("(c p) o -> p (c o)", p=128)
        )
        nc.gpsimd.dma_start(
            out=bv_bc, in_=bv.rearrange("s o -> o s").to_broadcast([128, DS])
        )

        # ---- engine-clock model (for exp/quantize/drain placement) ----
        eng_clk = {"a": 0.0, "d": 0.0}
        C_EA, C_ED = 1040.0, 1290.0  # exp per tick
        C_QA, C_QD = 600.0, 670.0  # qk quantize [128,512]
        C_VB = 670.0  # v bias (DVE only)
        C_NRM = 880.0  # normalize per pair (DVE only)
        C_DA, C_DD = 600.0, 670.0  # oproj drain half
        dve_exp_used = [0]

        def pick_eng(ca, cd):
            if eng_clk["a"] + ca <= eng_clk["d"] + cd:
                eng_clk["a"] += ca
                return "a"
            eng_clk["d"] += cd
            return "d"

        def ld_x1(sq, eng):
            t = px1.tile([64, 2, NKC, QB], FP8, tag="x8", name=f"x1_{sq}")
            eng.dma_start(
                out=t,
                in_=x1[:, :, sq * QB : (sq + 1) * QB].rearrange(
                    "p (j c) s -> p j c s", j=2
                ),
            )
            x1ts[sq] = t

        def ld_x2f(sq, eng):
            eng.dma_start(
                out=x2f_sb[:, :, :, sq * QB : (sq + 1) * QB],
                in_=x2f[:, :, sq * QB : (sq + 1) * QB].rearrange(
                    "p (j c) s -> p j c s", j=2
                ),
            )

        def ld_x2b(sq, eng):
            eng.dma_start(
                out=x2b_sb[:, :, sq * QB : (sq + 1) * QB],
                in_=x2b.rearrange("(c p) s -> p c s", p=128)[
                    :, :, sq * QB : (sq + 1) * QB
                ],
            )

        # fp8 DoubleRow projection chunk c of quarter sq -> fp8 qt8/kt8
        # (columns host-permuted so psum partitions land in the
        # scores-DoubleRow layout); bias-add+fp8-quantize on the less
        # loaded of ACT/DVE.
        # Cross-engine consumers (quantize / bias / drain / normalize /
        # transpose) are NOT emitted right after their producer matmuls:
        # engines execute in order, so a dep-waiting op at the head of
        # ACT/DVE would stall the exp stream behind it. Instead they are
        # queued here and emitted 1+ ticks later, when the producer is
        # (nearly) done. Safe with single-buffer psum scratch because
        # deferred thunks run BEFORE the tick's fill job allocates.
        deferred = defaultdict(list)
        cur_tick = [0]

        def qk_chunk(psp, which, sq, c):
            w_sb, dst, b_sb = (
                (wq8_sb, qt8, bq_sb) if which == 1 else (wk8_sb, kt8, bk_sb)
            )
            xt = x1ts[sq] if which == 1 else x2f_sb
            h, c2 = c // 2, c % 2
            ps = psp.tile([128, QB], F32, tag="pp", name="qkp")
            for kc in range(NKC):
                if which == 1:
                    mov = xt[:, :, kc, :]
                else:
                    mov = xt[:, :, kc, sq * QB : (sq + 1) * QB]
                nc.tensor.matmul(
                    ps,
                    w_sb[:, h, :, kc, c2 * 128 : (c2 + 1) * 128],
                    mov,
                    start=(kc == 0),
                    stop=(kc == NKC - 1),
                    perf_mode=DRow,
                )
            dsts = dst[:, c // 2, c % 2, sq * QB : (sq + 1) * QB]

            def quant():
                if pick_eng(C_QA, C_QD) == "a":
                    nc.scalar.activation(
                        dsts, ps, Ident, bias=b_sb[:, c : c + 1]
                    )
                else:
                    nc.vector.tensor_scalar_add(dsts, ps, b_sb[:, c : c + 1])

            return quant

        def v_chunk(psp, sq, sc):
            ps = psp.tile([128, QB], F32, tag="pp", name="vp")
            for kc in range(NKC):
                nc.tensor.matmul(
                    ps,
                    x2b_sb[:, kc, sq * QB + sc * 128 : sq * QB + (sc + 1) * 128],
                    wv_sb[:, kc, :],
                    start=(kc == 0),
                    stop=(kc == NKC - 1),
                )

            def vbias():
                eng_clk["d"] += C_VB
                nc.vector.tensor_add(
                    v_sb[:, sq * 4 + sc, :, :],
                    ps.rearrange("p (h d) -> p h d", h=HPC),
                    bv_bc.rearrange("p (h d) -> p h d", h=HPC),
                )

            return vbias

        # ---- load streams ----
        # DMA transfers serialize on the modeled (exclusive) DMA device, so
        # order IS the schedule. Scalar carries only the loads the first
        # scores need; everything else on sync. ACT/DVE never issue DMAs
        # (their 667ns seq dispatch would stall the exp stream).
        dma_ready = {}
        _dma_clk = [2000.0]

        def dma_ns(total_bytes, elem):
            lat = 2.0 if elem < 512 else 1.0
            return total_bytes / elem / 16.0 * max(elem * lat / 22.5, 7.0)

        def _track(name, total_bytes, elem, emit):
            emit()
            _dma_clk[0] += dma_ns(total_bytes, elem) + 800.0
            dma_ready[name] = _dma_clk[0]

        KB = 1024
        _track("wk8h", 256 * KB, 4096,
               lambda: nc.scalar.dma_start(out=wk8_sb[:, 0], in_=wk8[:, 0, :].rearrange("p (j c i) -> p j c i", j=2, c=NKC)))
        _track("x2f0", 512 * KB, 512, lambda: ld_x2f(0, nc.scalar))
        _track("wq8h", 256 * KB, 4096,
               lambda: nc.sync.dma_start(out=wq8_sb[:, 0], in_=wq8[:, 0, :].rearrange("p (j c i) -> p j c i", j=2, c=NKC)))
        _track("x1q0", 512 * KB, 512, lambda: ld_x1(0, nc.sync))
        _track("x2f1", 512 * KB, 512, lambda: ld_x2f(1, nc.sync))
        _track("x2f2", 512 * KB, 512, lambda: ld_x2f(2, nc.sync))
        _track("wv", 1024 * KB, 1024,
               lambda: nc.sync.dma_start(out=wv_sb, in_=wv.rearrange("(c p) o -> p c o", p=128)))
        _track("x2f3", 512 * KB, 512, lambda: ld_x2f(3, nc.sync))
        _track("x2b0", 1024 * KB, 1024, lambda: ld_x2b(0, nc.sync))
        _track("x2b1", 1024 * KB, 1024, lambda: ld_x2b(1, nc.sync))
        _track("x2b2", 1024 * KB, 1024, lambda: ld_x2b(2, nc.sync))

        # startup projection chunks in their own psum pool. Dummy matmuls
        # first ramp the PE clock out of its cold p-state.
        warm = pers.tile([128, 512], BF16)
        nc.gpsimd.memset(warm[:, :], 0.0)
        with tc.tile_pool(name="psA", bufs=4, space="PSUM") as psA:
            wps = psA.tile([128, 512], F32, tag="pp", name="warmp")
            for i in range(8):
                nc.tensor.matmul(
                    wps[0:1, :],
                    ones[:, 0:1],
                    warm[:, :],
                    start=(i == 0),
                    stop=(i == 7),
                )
            qk_chunk(psA, 2, 0, 0)()
            qk_chunk(psA, 2, 0, 1)()
            qk_chunk(psA, 1, 0, 0)()
            qk_chunk(psA, 1, 0, 1)()

        # ---- attention pools ----
        attn_ctx = ctx.enter_context(ExitStack())
        pso = attn_ctx.enter_context(tc.tile_pool(name="pso", bufs=1, space="PSUM"))
        pdn = attn_ctx.enter_context(tc.tile_pool(name="pdn", bufs=1, space="PSUM"))
        pex = attn_ctx.enter_context(tc.tile_pool(name="pex", bufs=EX_BUFS))
        pao = attn_ctx.enter_context(tc.tile_pool(name="pao", bufs=2))
        prd = attn_ctx.enter_context(tc.tile_pool(name="prd", bufs=2))
        pot = attn_ctx.enter_context(tc.tile_pool(name="pot", bufs=2))
        # created last so they can close first (LIFO), freeing 6 psum banks
        # for the tail oproj pipeline
        sc_ctx = ExitStack()
        psc = sc_ctx.enter_context(tc.tile_pool(name="psc", bufs=2, space="PSUM"))
        psp = sc_ctx.enter_context(tc.tile_pool(name="psp", bufs=2, space="PSUM"))

        ot_tiles = {}

        def oproj_half(mb, nt):
            if nt == 0:
                ot_tiles[mb] = pot.tile(
                    [128, D_MODEL], BF16, tag="ot", name="oti"
                )
            ot = ot_tiles[mb]
            ps = psp.tile([128, 512], F32, tag="pp", name="opp")
            for kc in range(4):
                nc.tensor.matmul(
                    ps,
                    aot[:, kc, mb * 128 : (mb + 1) * 128],
                    wo_sb[:, kc, nt * 512 : (nt + 1) * 512],
                    start=(kc == 0),
                    stop=(kc == 3),
                )

            def drain():
                if pick_eng(C_DA, C_DD) == "a":
                    nc.scalar.activation(
                        ot[:, nt * 512 : (nt + 1) * 512], ps, Copy
                    )
                else:
                    nc.vector.tensor_copy(ot[:, nt * 512 : (nt + 1) * 512], ps)
                if nt == 1:
                    nc.gpsimd.dma_start(
                        out=out[mb * 128 : (mb + 1) * 128, :], in_=ot
                    )

            return drain

        # ---- filler schedule: tick -> thunks ----
        # chunk c=(hslot,dj) of a quarter serves pairs 2*(c//2), 2*(c//2)+1.
        CQK, CV, COPH = 860, 1707, 860
        T0 = float(_os.environ.get("K_T0", 11000.0))
        TICK = float(_os.environ.get("K_TICK", 960.0))

        def r2t(ns):
            return max(0, int((ns - T0) / TICK) + 1)

        # jobs: (deadline_tick, ready_tick, pe_cost, thunk, tag). One heavy
        # job per tick, earliest-deadline-first among ready jobs. qt/kt
        # chunks MUST land by their deadline (scores would otherwise read
        # uninitialized sbuf = a real race); v jobs may slip (attn@V release
        # is gated on the v chunk being emitted, the backlog absorbs it).
        jobs = []
        for s in range(1, 4):
            for c in range(2):
                jobs.append(
                    (4 * s - 4 + c, r2t(dma_ready[f"x2f{s}"]), CQK,
                     lambda s=s, c=c: qk_chunk(psp, 2, s, c), None, 4 * s)
                )
        # pair 2 (tick 32+4s) reads BOTH dj chunks (c2 and c3) of hslot 1
        # c2/c3 need the half-1 weight loads dispatched at fill ticks 3/4:
        # ready >= 6 also orders the emission after those dma_starts.
        for s in range(4):
            rd = r2t(dma_ready[f"x2f{s}"])
            jobs.append(
                (28 + 4 * s, max(rd, 6), CQK,
                 lambda s=s: qk_chunk(psp, 2, s, 2), None, 32 + 4 * s)
            )
            jobs.append(
                (29 + 4 * s, max(rd, 6), CQK,
                 lambda s=s: qk_chunk(psp, 2, s, 3), None, 32 + 4 * s)
            )
        for c in (2, 3):
            jobs.append(
                (26 + c, max(r2t(dma_ready["x1q0"]), 6), CQK,
                 lambda c=c: qk_chunk(psp, 1, 0, c), None, 32)
            )
        for s in range(4):
            rv = r2t(max(dma_ready.get(f"x2b{s}", 0.0), dma_ready["wv"])) \
                if s < 3 else 28
            for sc in range(4):
                dl = max(4 * s + sc + AGE - 1, rv)
                jobs.append(
                    (dl, rv, CV,
                     lambda s=s, sc=sc: v_chunk(psp, s, sc), ("v", 4 * s + sc),
                     None)
                )
        for sq in range(1, 4):
            for c in range(4):
                jobs.append(
                    (64 * sq - 4 + c if c < 2 else 64 * sq + 26 + c,
                     64 * (sq - 1) + 6, CQK,
                     lambda sq=sq, c=c: qk_chunk(psp, 1, sq, c), None,
                     64 * sq if c < 2 else 64 * sq + 32)
                )
        # oproj halves: the source qb's last transpose is emitted when its
        # final attn@V leaves the backlog (qb end + AGE)
        # aot for block qb is complete once pair3's transposes are emitted:
        # release(qb,3,15) ~ 64qb+63+AGE, + NLAG (norm) + TLAG (tps) + margin
        for mb in range(12):
            for nt in range(2):
                jobs.append(
                    (1000 + 2 * mb + nt,
                     64 * (mb // 4 + 1) + AGE + TLAG + 4, COPH,
                     lambda mb=mb, nt=nt: oproj_half(mb, nt), None, None)
                )
        jobs.sort(key=lambda j: (j[0], j[1]))
        # Precompute the EDF tick assignment; qt/kt jobs must land by their
        # deadline, v jobs gate attn@V release below.
        v_done_tick = {}
        _pending = list(range(len(jobs)))
        assigned = {}
        for t in range(256):
            pick = None
            for idx in _pending:
                if jobs[idx][0] <= t or jobs[idx][1] <= t:
                    pick = idx
                    break
            if pick is not None:
                assigned[t] = pick
                _pending.remove(pick)
                tag = jobs[pick][4]
                if tag and tag[0] == "v":
                    v_done_tick[tag[1]] = t
                rt = jobs[pick][5]
                if rt is not None:
                    assert t <= rt - 3, (t, rt)
        assert not _pending, f"{len(_pending)} jobs unassigned"
        assert len(v_done_tick) == 16
        # zero-cost emissions (loads) at fixed ticks
        fill = defaultdict(list)
        fill[2].append(lambda: ld_x2b(3, nc.sync))
        fill[3].append(
            lambda: nc.sync.dma_start(out=wk8_sb[:, 1], in_=wk8[:, 1, :].rearrange("p (j c i) -> p j c i", j=2, c=NKC))
        )
        fill[4].append(
            lambda: nc.sync.dma_start(out=wq8_sb[:, 1], in_=wq8[:, 1, :].rearrange("p (j c i) -> p j c i", j=2, c=NKC))
        )
        fill[5].append(lambda: ld_x1(1, nc.sync))
        for sq in range(2, 4):
            fill[64 * (sq - 1) + 2].append(lambda sq=sq: ld_x1(sq, nc.sync))
        fill[20].append(
            lambda: nc.sync.dma_start(
                out=wo_sb, in_=wo.rearrange("(c p) o -> p c o", p=128)
            )
        )

        # ---- flat pipelined attention stream ----
        TICKS = [
            (qb, pair, kc)
            for qb in range(4)
            for pair in range(4)
            for kc in range(NSC)
        ]
        sc_tiles = {}
        ex_tiles = {}
        state = {}

        def emit_sc(t):
            qb, pair, kc = TICKS[t]
            q0 = qb * QB
            scp = psc.tile([128, 2, QB], F32, tag="sc", name="scp")
            for par in range(2):
                h = 2 * pair + par
                hb = 32 * (h % 4)
                nc.tensor.matmul(
                    scp[:, par, :],
                    kt8[hb : hb + 32, h // 4, :, kc * 128 : (kc + 1) * 128],
                    qt8[hb : hb + 32, h // 4, :, q0 : q0 + QB],
                    start=True,
                    stop=True,
                    perf_mode=DRow,
                    tile_position=(hb, 0),
                )
            sc_tiles[t] = scp

        def emit_exp(t, ready_ns):
            expt = pex.tile([128, 2, QB], BF16, tag="ex", name="ext")
            scp = sc_tiles.pop(t)
            fa = max(eng_clk["a"], ready_ns) + C_EA
            fd = max(eng_clk["d"], ready_ns) + C_ED
            if fd < fa and dve_exp_used[0] < DVE_EXP_MAX:
                dve_exp_used[0] += 1
                eng_clk["d"] = fd
                nc.vector.tensor_scalar(
                    expt[:, :, :].bitcast(I16), scp, AEXP, BEXP, Mult, Add
                )
            else:
                eng_clk["a"] = fa
                nc.scalar.activation(expt, scp, Exp, scale=float(SCALE))
            ex_tiles[t] = expt

        def emit_av(t):
            qb, pair, kc = TICKS[t]
            expt = ex_tiles.pop(t)
            if kc == 0:
                state["outp"] = pso.tile(
                    [128, 2, 4, DH], F32, tag="acc", name="outp"
                )
                if pair == 0:
                    state["den"] = pdn.tile([128, 32], F32, tag="dn", name="den")
                    state["rden"] = prd.tile([128, 32], F32, tag="rd", name="rden")
            outp = state["outp"]
            den = state["den"]
            for par in range(2):
                h = 2 * pair + par
                for qc in range(4):
                    exs = expt[:, par, qc * 128 : (qc + 1) * 128]
                    first = kc == 0 and par == 0 and qc == 0
                    last = kc == NSC - 1 and par == 1 and qc == 3
                    nc.tensor.matmul(
                        outp[:, par, qc, :],
                        exs,
                        v_sb[:, kc, h, :],
                        start=first,
                        stop=last,
                    )
                    di = pair * 8 + par * 4 + qc
                    nc.tensor.matmul(
                        den[:, di : di + 1],
                        exs,
                        ones[:, :],
                        start=first,
                        stop=last,
                    )
            if kc == NSC - 1:
                q0 = qb * QB
                rden = state["rden"]
                # reciprocal inline: it must execute before the next pair's
                # den accumulation group opens in the same psum tensor
                rsl = rden[:, pair * 8 : (pair + 1) * 8]
                nc.vector.reciprocal(rsl, den[:, pair * 8 : (pair + 1) * 8])
                ao = pao.tile([128, 4, 2, DH], BF16, tag="ao", name="ao")
                nc.vector.tensor_tensor(
                    out=ao[:, :, :, :],
                    in0=outp.rearrange("p a b d -> p b a d"),
                    in1=rsl.rearrange("p (a b) -> p b a", a=2).to_broadcast(
                        [128, 4, 2, DH]
                    ),
                    op=Mult,
                )
                eng_clk["d"] += C_NRM

                def tps():
                    for qc in range(4):
                        nc.sync.dma_start_transpose(
                            aot[:, pair, q0 + qc * 128 : q0 + (qc + 1) * 128],
                            ao[:, qc, :, :],
                        )

                deferred[cur_tick[0] + TLAG].append(tps)

        # Greedy emission: track modeled PE/ACT/DVE clocks; defer attn@V
        # work (bounded backlog) and drain it age-based so the ex pool never
        # starves the exp stream. Never release an attn@V whose v chunk
        # hasn't been emitted yet (emission order defines dependency order).
        C_SC, C_AV = 213.0, 220.0
        pe_t = 5500.0  # first matmul lands after the startup DMA chain
        eng_clk["a"] = eng_clk["d"] = 7000.0
        sc_done = {}
        backlog = []

        emit_sc(0)
        sc_done[0] = pe_t = pe_t + C_SC
        emit_sc(1)
        sc_done[1] = pe_t = pe_t + C_SC
        for t in range(256):
            cur_tick[0] = t
            # deferred producers-consumers first: their inputs are ~done, and
            # queue position ahead of exp(t) lets vbias/quant unblock the
            # attn@V -> ex-pool chain instead of sitting behind a 1.2us exp
            for th in deferred.pop(t, ()):
                th()
            emit_exp(t, sc_done[t] + 100.0)
            backlog.append(t)
            for f in fill[t]:
                f()
            if t in assigned:
                dl, rd, cost, th, tag, rt = jobs[assigned[t]]
                post = th()
                pe_t += cost
                if post is not None:
                    # the quantize must be emitted before the scores reading
                    # its output region (emitted at tick rt-2, sc phase)
                    lag = QLAG if rt is None else max(1, min(QLAG, rt - 2 - t))
                    deferred[t + lag].append(post)
            # attn@V after the job: these small matmuls overlap the job's
            # psum-drain latency so back-to-back fills don't bubble PE.
            while backlog and (
                len(backlog) > MAX_BACKLOG
                or (len(backlog) > D_MIN and t - backlog[0] >= AGE)
            ):
                qbu, pairu, kcu = TICKS[backlog[0]]
                if qbu == 0 and v_done_tick[kcu] + QLAG > t:
                    break
                emit_av(backlog.pop(0))
                pe_t += C_AV
            # exp-gated score matmul last, so jobs/attn@V never sit behind
            # the gate in the PE queue
            if t + 2 < 256:
                emit_sc(t + 2)
                pe_t += C_SC
                sc_done[t + 2] = pe_t
        # psc/psp are done once the in-loop deferred quantizes/drains flush;
        # release their 6 banks for the tail oproj pipeline
        sc_ctx.close()

        # ---- tail: drain backlog while accumulating the last oproj blocks'
        # first three contraction steps (aot pairs 0-2 are long since
        # transposed); pair 3's chunk + drains follow the final transposes.
        psD_ctx = ExitStack()
        psD = psD_ctx.enter_context(tc.tile_pool(name="psD", bufs=4, space="PSUM"))
        tail_ps = {}
        tail_ot = {}

        def tail_phaseA(mb, nt):
            if nt == 0:
                tail_ot[mb] = pot.tile(
                    [128, D_MODEL], BF16, tag="ot", name="otd"
                )
            ps = psD.tile([128, 512], F32, tag="pf", name="opd")
            tail_ps[(mb, nt)] = ps
            for kc in range(3):
                nc.tensor.matmul(
                    ps,
                    aot[:, kc, mb * 128 : (mb + 1) * 128],
                    wo_sb[:, kc, nt * 512 : (nt + 1) * 512],
                    start=(kc == 0),
                    stop=False,
                )

        def tail_finish(mb, nt):
            ps = tail_ps.pop((mb, nt))
            ot = tail_ot[mb]
            nc.tensor.matmul(
                ps,
                aot[:, 3, mb * 128 : (mb + 1) * 128],
                wo_sb[:, 3, nt * 512 : (nt + 1) * 512],
                start=False,
                stop=True,
            )
            if nt == 0:
                nc.scalar.activation(ot[:, 0:512], ps, Copy)
            else:
                nc.vector.tensor_copy(ot[:, 512:1024], ps)
                nc.sync.dma_start(out=out[mb * 128 : (mb + 1) * 128, :], in_=ot)

        for u in backlog:
            cur_tick[0] += 1
            emit_av(u)
            for th in deferred.pop(cur_tick[0], ()):
                th()
        # phase-A after the backlog (PE would idle during the final
        # norm/transposes otherwise; before it, it would delay them)
        for mb, nt in ((12, 0), (12, 1), (13, 0), (13, 1)):
            tail_phaseA(mb, nt)
        while deferred:
            t = min(deferred)
            cur_tick[0] = max(cur_tick[0], t)
            for th in deferred.pop(t):
                th()
        # final transposes are emitted by now; finish 12/13, pipeline 14/15
        tail_finish(12, 0)
        tail_finish(12, 1)
        tail_phaseA(14, 0)
        tail_phaseA(14, 1)
        tail_finish(13, 0)
        tail_finish(13, 1)
        tail_phaseA(15, 0)
        tail_phaseA(15, 1)
        tail_finish(14, 0)
        tail_finish(14, 1)
        tail_finish(15, 0)
        tail_finish(15, 1)
        psD_ctx.close()
        attn_ctx.close()


_NC_CACHE = []


def _build():
    if _NC_CACHE:
        return _NC_CACHE[0]
    nc = bacc.Bacc(None, target_bir_lowering=False, debug=False)
    x1 = nc.dram_tensor("x1", [64, 16, S], FP8, kind="ExternalInput")
    x2f = nc.dram_tensor("x2f", [64, 16, S], FP8, kind="ExternalInput")
    x2b = nc.dram_tensor("x2b", [D_IN, S], BF16, kind="ExternalInput")
    wq8 = nc.dram_tensor("wq8", [64, 2, 4096], FP8, kind="ExternalInput")
    wk8 = nc.dram_tensor("wk8", [64, 2, 4096], FP8, kind="ExternalInput")
    wv = nc.dram_tensor("wv", [D_IN, DS], BF16, kind="ExternalInput")
    wo = nc.dram_tensor("wo", [DS, D_MODEL], BF16, kind="ExternalInput")
    bq = nc.dram_tensor("bq", [DS, 1], F32, kind="ExternalInput")
    bk = nc.dram_tensor("bk", [DS, 1], F32, kind="ExternalInput")
    bv = nc.dram_tensor("bv", [DS, 1], F32, kind="ExternalInput")
    out = nc.dram_tensor("out", [S, D_MODEL], BF16, kind="ExternalOutput")
    with tile.TileContext(nc) as tc:
        _kernel_body(
            nc,
            tc,
            aps=(
                x1[:, :, :],
                x2f[:, :, :],
                x2b[:, :],
                wq8[:, :, :],
                wk8[:, :, :],
                wv[:, :],
                wo[:, :],
                bq[:, :],
                bk[:, :],
                bv[:, :],
                out[:, :],
            ),
        )
    nc.compile()
    _NC_CACHE.append(nc)
    return nc


def _run(inputs, trace=False, **kw):
    import ml_dtypes

    nc = _build()
    F8 = ml_dtypes.float8_e4m3
    BF = ml_dtypes.bfloat16
    f32 = lambda a: np.ascontiguousarray(np.asarray(a, dtype=np.float32))
    perm = _col_perm()
    X1, X2 = (
        np.asarray(inputs["X1"], np.float32),
        np.asarray(inputs["X2"], np.float32),
    )
    Wq, Wk = np.asarray(inputs["Wq"], np.float32), np.asarray(
        inputs["Wk"], np.float32
    )
    Wv, Wo = np.asarray(inputs["Wv"], np.float32), np.asarray(
        inputs["Wo"], np.float32
    )
    bqf, bkf = (
        np.asarray(inputs["bq"], np.float32),
        np.asarray(inputs["bk"], np.float32),
    )

    def to_x8(Xb):  # [S, D_IN] -> [64, 16, S] fp8, d = 128*kc + 64*j + p
        a = Xb.T.reshape(NKC, 2, 64, S).transpose(2, 1, 0, 3)
        return np.ascontiguousarray(a.reshape(64, 16, S)).astype(F8)

    def to_w8(Ws):  # [D_IN, DS] (col-perm'd) -> [64, 2, 4096]
        a = Ws.reshape(NKC, 2, 64, 2, 256).transpose(2, 3, 1, 0, 4)
        return np.ascontiguousarray(a.reshape(64, 2, 4096)).astype(F8)

    in_maps = []
    for c in range(8):
        b, hf = c // 2, c % 2
        sl = slice(hf * DS, (hf + 1) * DS)
        wq_s, wk_s = Wq[:, sl][:, perm], Wk[:, sl][:, perm]
        in_maps.append(
            {
                "x1": to_x8(X1[b]),
                "x2f": to_x8(X2[b]),
                "x2b": np.ascontiguousarray(X2[b].T).astype(BF),
                "wq8": to_w8(wq_s),
                "wk8": to_w8(wk_s),
                "wv": np.ascontiguousarray(Wv[:, sl]).astype(BF),
                "wo": np.ascontiguousarray(Wo[sl, :]).astype(BF),
                "bq": np.ascontiguousarray(bqf[sl][perm]).reshape(DS, 1),
                "bk": np.ascontiguousarray(bkf[sl][perm]).reshape(DS, 1),
                "bv": f32(inputs["bv"])[sl].reshape(DS, 1),
            }
        )
    res = run_bass_kernel_spmd(nc, in_maps, list(range(8)), trace=trace, **kw)
    parts = [np.asarray(res.results[c]["out"], np.float32) for c in range(8)]
    bo = f32(inputs["bo"])
    full = np.stack(
        [parts[2 * b] + parts[2 * b + 1] + bo[None, :] for b in range(B)]
    )
    return full.astype(np.float32), res


def kernel(**inputs):
    out, _ = _run(inputs, trace=False)
    return out
